# revision 12
# baseline (speedup 1.0000x reference)
"""AttnBlock on 8 TRN2 NeuronCores -- F(4,3) x-Winograd, folded-proj variant.

Same conv math as the F(4,3) baseline (host-side input/weight transforms,
fp8 DoubleRow GEMMs with fp32 PSUM, v produced pre-transposed by swapping
conv operands, deferred softmax normalization), restructured for overlap:

  - the 1x1 proj conv is folded into the v-conv weights on the host
    (both are linear maps over channels), so attn@v emits the final
    output channels directly: no hN intermediate, no proj GEMM stage.
    The tiny w_proj magnitudes (~1e-7) are scaled up by 2^26 into fp8
    range; the 2^-20 net factor rides the reciprocal broadcast.
  - merged matmuls: the two per-core samples share one qk matmul
    (rhs [P, i, s, 256], the ky window is contiguous in the flat (y,t)
    dim) and the two v-conv output halves share one matmul (rhs
    [P, i, co512]). Halves MATMUL+LDWEIGHTS instruction count and
    widens the A^T drain from 256 to 512 columns (fewer, cheaper DVE
    ops). PSUM banks hold one winograd plane for both samples/halves.
  - ONE PSUM tile pool (8 one-bank bufs, single tag) spans conv and
    attention: pool release/realloc at the phase boundary is a full
    barrier (released-zone alloc deps) that idled the PE ~8us and
    re-throttled HAM to K=4/8 for the attention phase.
  - the v-conv units are WOVEN between scores chunks: scores are paced
    by the ScalarE exp ACTIVATEs (~1.25us per chunk vs 0.97us of MM),
    so pure scores blocks stall the PE; v matmuls fill those slots.
  - initial DMAs: the gpsimd software queue sustains ~200+ GB/s, the
    sync/scalar hardware queues only ~40 GB/s. The conv-critical
    stream (u0 per-v chunks + v_in tiles in consumption order, then
    u1..u7) rides gpsimd; the slow queues prefetch the v-conv weights
    (not needed until ~70us in) and later carry the output tiles.
  - drain staging copies split ScalarE (c1) / DVE (c3).
"""

import numpy as np
import ml_dtypes

import concourse.bass as bass
import concourse.tile as tile
from concourse import bacc, mybir
from concourse.bass_utils import run_bass_kernel_spmd

P = 128
B, C, H, W = 16, 512, 32, 32
NCORES = 8
S = B // NCORES
HP = WP = H + 2
NPIX = H * W
CC = C // P
QKCH = 2 * CC
NT = 2
NTILE = 512
TX = W // 4          # 8 x-tiles
NP6 = 6              # winograd positions
KY = 3
NQK = H * TX         # 256: per-sample N for q/k GEMMs
VTC = 2              # v-conv t-chunks per sample (256 t / 128)
SU = 32.0            # q/k weight scale into fp8
SWV = float(2 ** 26)  # folded v*proj weight scale into fp8
VEMIT = 1.0 / 64     # v-conv PSUM -> fp8 emit scale
RSC = 2.0 ** -20     # net 1/(SWV*VEMIT) rides the reciprocal broadcast

BF16 = mybir.dt.bfloat16
F32 = mybir.dt.float32
F8 = mybir.dt.float8e4
EXP = mybir.ActivationFunctionType.Exp
DR = mybir.MatmulPerfMode.DoubleRow
MULT = mybir.AluOpType.mult
ADD = mybir.AluOpType.add

TRACE = False
LAST_EXEC_NS = None

_CACHED = {}

UCH = NP6 * KY * 2 * 2 * P  # bytes/partition of one q/k weight tile
RFL = HP * TX               # flat (y,t) extent per (i,s)


def build_nc(with_bias=True):
    nc = bacc.Bacc()
    vw_d = nc.declare_dram_parameter("vw", [NP6, 2, P, 2 * S * HP * TX], F8,
                                     isOutput=False)
    uw_d = nc.declare_dram_parameter("uw", [QKCH, P, UCH], F8, isOutput=False)
    uv_d = nc.declare_dram_parameter("uv", [P, NP6 * KY * 2 * 2 * CC * P], F8,
                                     isOutput=False)
    bqkv_d = nc.declare_dram_parameter("bqkv", [QKCH, P], BF16, isOutput=False)
    out_d = nc.declare_dram_parameter("out", [S, P, CC, NPIX], BF16, isOutput=True)

    with tile.TileContext(nc) as tc:
        with (
            tc.tile_pool(name="const", bufs=1) as constp,
            tc.tile_pool(name="resid", bufs=1) as resid,
            tc.tile_pool(name="stage", bufs=2) as stage,
        ):
            ones8 = constp.tile([P, 2, 16], F8, name="ones8")
            nc.vector.memset(ones8, 1.0)
            ones_nt = constp.tile([1, NTILE], BF16, name="ones_nt")
            nc.vector.memset(ones_nt, 1.0)
            ones_colT = constp.tile([1, P], BF16, name="ones_colT")
            nc.vector.memset(ones_colT, RSC)

            v_in = {}
            v_qk = {}   # [p, i, s, flat(y,t)] for merged-sample qk matmuls
            v_fl = {}
            for v in range(NP6):
                for j in range(2):
                    t = resid.tile([P, 2 * S * RFL], F8, tag="vin",
                                   bufs=NP6 * 2, name=f"vin_{v}_{j}")
                    v_in[(v, j)] = t
                    v_qk[(v, j)] = t.rearrange("p (i s r) -> p i s r",
                                               i=2, s=S)
                    v_fl[(v, j)] = t.rearrange("p (i r) -> p i r", i=2)

            bias_sb = constp.tile([1, QKCH, P], BF16, name="bias_sb")
            if with_bias:
                nc.sync.dma_start(bias_sb, bqkv_d[:])

            qk8 = {}
            for s in range(S):
                for w8 in ("q", "k"):
                    for j in range(2):
                        qk8[(s, w8, j)] = resid.tile(
                            [P, 2, NPIX], F8, tag="qk8", bufs=S * 4,
                            name=f"{w8}8_{s}_{j}")
            # vT pair tiles: key (s, tc, jp); [t-part, jj%2, co],
            # pixel m = 512*tc + 4p + (2*jp + jj%2); co = folded proj-out ch
            vT8_sb = {}
            for s in range(S):
                for tcn in range(VTC):
                    for jp in range(2):
                        vT8_sb[(s, tcn, jp)] = resid.tile(
                            [P, 2, C], F8, tag="vt", bufs=S * VTC * 2,
                            name=f"vt8_{s}_{tcn}_{jp}")

            uvpool = tc.alloc_tile_pool(name="uv", bufs=1)
            uv_sb = uvpool.tile([P, NP6 * KY * 2 * 2 * CC * P], F8, tag="uv",
                                bufs=1, name="uv")
            uv_vw = uv_sb.rearrange("p (slot i co) -> p slot i co",
                                    slot=NP6 * KY * 2, i=2, co=C)

            upool = tc.alloc_tile_pool(name="uw", bufs=1)
            u_sb = {}
            u_vw = {}
            for oc in range(QKCH):
                t = upool.tile([P, UCH], F8, tag="u", bufs=4, name=f"u_{oc}")
                u_sb[oc] = t
                u_vw[oc] = t.rearrange("p (slot i co) -> p slot i co",
                                       slot=NP6 * KY * 2, i=2, co=P)

            UVC = KY * 2 * 2 * P   # bytes/partition of one v-position of uw
            UVV = KY * 2 * 2 * C   # bytes/partition of one v-position of uv
            nc.sync.dma_start(v_in[(0, 0)], vw_d[0, 0])
            nc.sync.dma_start(v_in[(0, 1)], vw_d[0, 1])
            nc.gpsimd.dma_start(u_sb[0][:, 0:UVC], uw_d[0][:, 0:UVC])
            nc.gpsimd.dma_start(u_sb[0][:, UVC:2 * UVC], uw_d[0][:, UVC:2 * UVC])
            for v in range(1, NP6):
                for j in range(2):
                    nc.gpsimd.dma_start(v_in[(v, j)], vw_d[v, j])
                if v < NP6 - 1:
                    nc.gpsimd.dma_start(
                        u_sb[0][:, (v + 1) * UVC:(v + 2) * UVC],
                        uw_d[0][:, (v + 1) * UVC:(v + 2) * UVC])
            for v in range(2, 4):
                nc.scalar.dma_start(uv_sb[:, v * UVV:(v + 1) * UVV],
                                    uv_d[:, v * UVV:(v + 1) * UVV])
            for v in range(4, NP6):
                nc.sync.dma_start(uv_sb[:, v * UVV:(v + 1) * UVV],
                                  uv_d[:, v * UVV:(v + 1) * UVV])
            for oc in (1, 2):
                nc.gpsimd.dma_start(u_sb[oc], uw_d[oc])
            for v in range(2):
                nc.gpsimd.dma_start(uv_sb[:, v * UVV:(v + 1) * UVV],
                                    uv_d[:, v * UVV:(v + 1) * UVV])
            for oc in range(3, QKCH):
                nc.gpsimd.dma_start(u_sb[oc], uw_d[oc])

            def drain6(pl, emit):
                # pl[v] = [P, 512] PSUM views of the six A^T input planes;
                # rows {1,1,1,1,1,0; 0,1,-1,2,-2,0; 0,1,1,4,4,0;
                # 0,1,-1,8,-8,1}; ScalarE stages M1, DVE stages M3 (one
                # PSUM operand per DVE op); emit(j, o_bf16[P,512])
                n_ = NTILE
                c1 = stage.tile([P, n_], F32, tag="c1", name=f"c1_{drain6.n}")
                c3 = stage.tile([P, n_], F32, tag="c3", name=f"c3_{drain6.n}")
                sp = stage.tile([P, n_], F32, tag="sp", name=f"sp_{drain6.n}")
                sm = stage.tile([P, n_], F32, tag="sm", name=f"sm_{drain6.n}")
                p34 = stage.tile([P, n_], F32, tag="p34", name=f"p34_{drain6.n}")
                m34 = stage.tile([P, n_], F32, tag="m34", name=f"m34_{drain6.n}")
                t0 = stage.tile([P, n_], F32, tag="t0", name=f"t0_{drain6.n}")
                t3 = stage.tile([P, n_], F32, tag="t3", name=f"t3_{drain6.n}")
                o = [stage.tile([P, n_], BF16, tag=f"o{j}",
                                name=f"o{j}_{drain6.n}") for j in range(4)]
                drain6.n += 1
                nc.scalar.copy(out=c1, in_=pl[1])
                nc.vector.tensor_copy(out=c3, in_=pl[3])
                nc.vector.tensor_add(sp, c1, pl[2])
                nc.vector.tensor_sub(sm, c1, pl[2])
                nc.vector.tensor_add(p34, c3, pl[4])
                nc.vector.tensor_sub(m34, c3, pl[4])
                nc.vector.tensor_add(t0, pl[0], sp)
                nc.vector.tensor_add(o[0], t0, p34)
                nc.vector.scalar_tensor_tensor(o[1], m34, 2.0, sm,
                                               op0=MULT, op1=ADD)
                nc.vector.scalar_tensor_tensor(o[2], p34, 4.0, sp,
                                               op0=MULT, op1=ADD)
                nc.vector.scalar_tensor_tensor(t3, m34, 8.0, pl[5],
                                               op0=MULT, op1=ADD)
                nc.vector.tensor_add(o[3], t3, sm)
                for j in range(4):
                    emit(j, o[j])
            drain6.n = 0

            # ONE PSUM pool for the whole kernel: 8 rotating 1-bank tiles.
            mpool = tc.alloc_tile_pool(name="mconv", bufs=1, space="PSUM")

            def bank(name):
                return mpool.tile([P, 2, 256], F32, tag="m", bufs=8, name=name)

            # q/k conv: plane v in one bank [P, s(2), 256], both samples per
            # matmul (the ky window is contiguous in the flat (y,t) dim)
            def qk_group(oc):
                pls = [bank(f"m_{oc}_{v}") for v in range(NP6)]
                plv = [t.rearrange("p a n -> p (a n)") for t in pls]
                if with_bias:
                    nc.tensor.matmul(plv[1], lhsT=bias_sb[0:1, oc],
                                     rhs=ones_nt[:, 0:NTILE],
                                     start=True, stop=False)
                for v in range(NP6):
                    for ky in range(KY):
                        for j in range(2):
                            nc.tensor.matmul(
                                pls[v],
                                lhsT=u_vw[oc][:, (v * KY + ky) * 2 + j],
                                rhs=v_qk[(v, j)][:, :, :,
                                                 ky * TX:ky * TX + H * TX],
                                start=((v != 1 or not with_bias)
                                       and ky == 0 and j == 0),
                                stop=(ky == KY - 1 and j == 1),
                                perf_mode=DR)
                if oc < CC:
                    w8, ii = "q", oc
                else:
                    w8, ii = "k", oc - CC

                def emit_qk(j, o, w8=w8, ii=ii):
                    ov = o.rearrange("p (s y tx) -> p s y tx", s=S, tx=TX)
                    for s in range(S):
                        dvv = qk8[(s, w8, ii // 2)].rearrange(
                            "p a (y tx four) -> p a y tx four", four=4, tx=TX)
                        nc.scalar.copy(dvv[:, ii % 2, :, :, j], ov[:, s])
                drain6(plv, emit_qk)

            # v conv: operands swapped -> M = [t(128), co]; both co halves
            # share one matmul (rhs [P, i, co512]); weights carry the folded
            # 1x1 proj. Generator yields after each (v, ky) unit so scores
            # chunks can be woven between units.
            def v_units():
                for s, tcn in [(s, t) for s in range(S) for t in range(VTC)]:
                    pls = [bank(f"mv_{s}_{tcn}_{v}") for v in range(NP6)]
                    plv = [t.rearrange("p a n -> p (a n)") for t in pls]
                    for v in range(NP6):
                        for ky in range(KY):
                            off = s * RFL + (16 * tcn + ky) * TX
                            for j in range(2):
                                nc.tensor.matmul(
                                    plv[v],
                                    lhsT=v_fl[(v, j)][:, :, off:off + P],
                                    rhs=uv_vw[:, (v * KY + ky) * 2 + j],
                                    start=(ky == 0 and j == 0),
                                    stop=(ky == KY - 1 and j == 1),
                                    perf_mode=DR)
                            yield v

                    def emit_v(j, o, s=s, tcn=tcn):
                        nc.scalar.mul(vT8_sb[(s, tcn, j // 2)][:, j % 2, :],
                                      o, VEMIT)
                    drain6(plv, emit_v)

            for oc in range(QKCH):
                qk_group(oc)

            upool.release()

            # ---- attention (reuses the released u-weight SBUF zone; the
            # zone dep is on the last qk LDWEIGHTS, already behind us) ----
            attn = tc.alloc_tile_pool(name="attn", bufs=1)
            exps8 = {}
            for s in range(S):
                for tcn in range(VTC):
                    for jp in range(2):
                        exps8[(s, tcn, jp)] = attn.tile(
                            [P, 2, NPIX], F8, tag="exps",
                            bufs=S * VTC * 2, name=f"exps_{s}_{tcn}_{jp}")
            r_sb = {}
            r_bf = {}
            rbc = {}
            for s in range(S):
                r_sb[s] = attn.tile([1, NPIX], F32, tag="r", bufs=S,
                                    name=f"r_{s}")
                r_bf[s] = attn.tile([1, NPIX], BF16, tag="rb16", bufs=S,
                                    name=f"rb16_{s}")

            def scores_chunk(s, tcn, jp, j2):
                jj = jp * 2 + j2
                ps_nt = [bank(f"ps_sc_{s}_{tcn}_{jj}_{nt}") for nt in range(NT)]
                for j in range(2):
                    kv = qk8[(s, "k", j)].rearrange(
                        "p a (m four) -> p a m four", four=4)
                    lhsT = kv[:, :, P * tcn:P * (tcn + 1), jj]
                    for nt in range(NT):
                        nc.tensor.matmul(
                            ps_nt[nt].rearrange("p a n -> p (a n)"),
                            lhsT=lhsT,
                            rhs=qk8[(s, "q", j)][
                                :, :, nt * NTILE:(nt + 1) * NTILE],
                            start=(j == 0), stop=(j == 1),
                            perf_mode=DR)
                for nt in range(NT):
                    nc.scalar.activation(
                        exps8[(s, tcn, jp)][:, j2, nt * NTILE:(nt + 1) * NTILE],
                        ps_nt[nt].rearrange("p a n -> p (a n)"),
                        EXP, scale=float(C) ** -0.5 / (SU * SU))

            def sums_block(s):
                keys = [(tcn, jp) for tcn in range(VTC) for jp in range(2)]
                for nt in range(NT):
                    ps_sum = bank(f"ps_sum_{s}_{nt}")
                    row = ps_sum.rearrange("p a n -> p (a n)")[0:1, :]
                    for ki, (tcn, jp) in enumerate(keys):
                        nc.tensor.matmul(
                            row, lhsT=ones8[:, :, 0:1],
                            rhs=exps8[(s, tcn, jp)][:, :,
                                                    nt * NTILE:(nt + 1) * NTILE],
                            start=(ki == 0), stop=(ki == len(keys) - 1),
                            perf_mode=DR)
                    nc.vector.reciprocal_approx_fast(
                        out=r_sb[s][:, nt * NTILE:(nt + 1) * NTILE], in_=row)
                nc.scalar.copy(out=r_bf[s], in_=r_sb[s])

            def rbc_block(s):
                for nt in range(NT):
                    ps_b = bank(f"ps_rb_{s}_{nt}")
                    pv = ps_b.rearrange("p a n -> p (a n)")
                    nc.tensor.matmul(pv, lhsT=ones_colT,
                                     rhs=r_bf[s][:, nt * NTILE:(nt + 1) * NTILE],
                                     start=True, stop=True)
                    rb = attn.tile([P, NTILE], F32, tag="rbc", bufs=2 * S,
                                   name=f"rbc_{s}_{nt}")
                    nc.scalar.copy(out=rb, in_=ps_b)
                    rbc[(s, nt)] = rb

            def out_block(s, och):
                # attn@v with proj-folded v: emits final output channels
                keys = [(tcn, jp) for tcn in range(VTC) for jp in range(2)]
                ps_h = [bank(f"ps_h_{s}_{och}_{nt}") for nt in range(NT)]
                for ki, (tcn, jp) in enumerate(keys):
                    lhsT = vT8_sb[(s, tcn, jp)][:, :, och * P:(och + 1) * P]
                    for nt in range(NT):
                        nc.tensor.matmul(
                            ps_h[nt].rearrange("p a n -> p (a n)"), lhsT=lhsT,
                            rhs=exps8[(s, tcn, jp)][
                                :, :, nt * NTILE:(nt + 1) * NTILE],
                            start=(ki == 0), stop=(ki == len(keys) - 1),
                            perf_mode=DR)
                for nt in range(NT):
                    ot = attn.tile([P, NTILE], BF16, tag="ost", bufs=4,
                                   name=f"ot_{s}_{och}_{nt}")
                    nc.vector.tensor_mul(
                        out=ot, in0=ps_h[nt].rearrange("p a n -> p (a n)"),
                        in1=rbc[(s, nt)])
                    eng = nc.sync if (och + nt) % 2 == 0 else nc.scalar
                    eng.dma_start(
                        out_d[s, :, och, nt * NTILE:(nt + 1) * NTILE], ot)

            # weave: the 72 v-conv (v,ky) units fill the PE while the
            # ScalarE exp ACTIVATEs pace the 16 scores chunks
            vu = v_units()

            def take(n):
                for _ in range(n):
                    if next(vu, None) is None:
                        break

            sc_keys = [(s, tcn, jp, j2) for s in range(S) for tcn in range(VTC)
                       for jp in range(2) for j2 in range(2)]
            for c, (s, tcn, jp, j2) in enumerate(sc_keys):
                take(5 if c % 2 else 4)
                scores_chunk(s, tcn, jp, j2)
                if c == 8:
                    sums_block(0)
                if c == 10:
                    rbc_block(0)
                if c >= 12:
                    out_block(0, c - 12)
            take(100)

            sums_block(1)
            rbc_block(1)
            out_block(1, 0)
            out_block(1, 1)
            out_block(1, 2)
            out_block(1, 3)

            mpool.release()
            attn.release()
            uvpool.release()

    nc.finalize()
    return nc


BT43 = np.array([
    [4, 0, -5, 0, 1, 0],
    [0, -4, -4, 1, 1, 0],
    [0, 4, -4, -1, 1, 0],
    [0, -2, -1, 2, 1, 0],
    [0, 2, -1, -2, 1, 0],
    [0, 4, 0, -5, 0, 1]], np.float32)
G43 = np.array([
    [1 / 4, 0, 0],
    [-1 / 6, -1 / 6, -1 / 6],
    [-1 / 6, 1 / 6, -1 / 6],
    [1 / 24, 1 / 12, 1 / 6],
    [1 / 24, -1 / 12, 1 / 6],
    [0, 0, 1]], np.float32)


def prep_inputs(x, w_qkv, b_qkv, w_proj):
    e4 = ml_dtypes.float8_e4m3
    xpad = np.zeros((B, C, HP, WP), np.float32)
    xpad[:, :, 1:H + 1, 1:W + 1] = x

    taps = np.stack([xpad[:, :, :, a:a + 4 * TX:4][:, :, :, :TX]
                     for a in range(6)])          # [6, B, C, HP, TX]
    V = np.tensordot(BT43, taps, axes=([1], [0]))  # [6, B, C, HP, TX]
    vw = np.ascontiguousarray(
        V.reshape(NP6, B, 2, 2, P, HP, TX)
        .transpose(0, 2, 4, 3, 1, 5, 6)).astype(e4)  # [6, 2, P, 2, B, HP, TX]

    u6qk = np.tensordot(w_qkv[:2 * C] * SU, G43, axes=([3], [1]))
    uw = np.ascontiguousarray(
        u6qk.reshape(QKCH, P, 2, 2, P, KY, NP6)
        .transpose(0, 4, 6, 5, 2, 3, 1)
        .reshape(QKCH, P, UCH)).astype(e4)
    # fold the 1x1 proj into the v weights: conv(x, Wp @ wv) == proj(conv(x, wv))
    u6v = np.tensordot(w_qkv[2 * C:], G43, axes=([3], [1]))  # [vc, ci, ky, 6]
    u6vp = np.tensordot(w_proj[:, :, 0, 0], u6v, axes=([1], [0])) * SWV
    uv = np.ascontiguousarray(
        u6vp.reshape(C, 2, 2, P, KY, NP6)
        .transpose(3, 5, 4, 1, 2, 0)
        .reshape(P, NP6 * KY * 2 * 2 * C)).astype(e4)
    bqkv = np.ascontiguousarray((b_qkv[:2 * C] * SU).reshape(QKCH, P)).astype(
        ml_dtypes.bfloat16)
    return vw, uw, uv, bqkv


def kernel(x, w_qkv, b_qkv, w_proj, b_proj, gn_gamma=None, gn_beta=None):
    global LAST_EXEC_NS
    x = np.asarray(x, np.float32)
    w_qkv = np.asarray(w_qkv, np.float32)
    b_qkv = np.asarray(b_qkv, np.float32)
    w_proj = np.asarray(w_proj, np.float32)
    b_proj = np.asarray(b_proj, np.float32)

    with_bias = bool(np.any(b_qkv[:2 * C]))
    if with_bias not in _CACHED:
        _CACHED[with_bias] = build_nc(with_bias=with_bias)
    nc = _CACHED[with_bias]

    vw, uw, uv, bqkv = prep_inputs(x, w_qkv, b_qkv, w_proj)

    in_maps = []
    for core in range(NCORES):
        sl = slice(core * S, (core + 1) * S)
        in_maps.append({
            "vw": np.ascontiguousarray(vw[:, :, :, :, sl]).reshape(NP6, 2, P, -1),
            "uw": uw,
            "uv": uv,
            "bqkv": bqkv,
        })

    res = run_bass_kernel_spmd(nc, in_maps, list(range(NCORES)), trace=TRACE)
    LAST_EXEC_NS = res.exec_time_ns
    h = np.stack([np.asarray(res.results[c]["out"], np.float32)
                  for c in range(NCORES)])
    h = h.reshape(B, P, CC, NPIX).transpose(0, 2, 1, 3).reshape(B, C, H, W)
    out = x + h + b_proj[None, :, None, None]
    # v-bias passes through softmax-weighting as a constant channel offset
    bv = b_qkv[2 * C:]
    if np.any(bv):
        out = out + (w_proj[:, :, 0, 0] @ bv)[None, :, None, None]
    return np.ascontiguousarray(out).astype(np.float32, copy=False)


# revision 13
# speedup vs baseline: 1.0087x; 1.0087x over previous
"""AttnBlock on 8 TRN2 NeuronCores -- F(4,3) x-Winograd, folded-proj variant.

Same conv math as the F(4,3) baseline (host-side input/weight transforms,
fp8 DoubleRow GEMMs with fp32 PSUM, v produced pre-transposed by swapping
conv operands, deferred softmax normalization), restructured for overlap:

  - the 1x1 proj conv is folded into the v-conv weights on the host
    (both are linear maps over channels), so attn@v emits the final
    output channels directly: no hN intermediate, no proj GEMM stage.
    The tiny w_proj magnitudes (~1e-7) are scaled up by 2^26 into fp8
    range; the 2^-20 net factor rides the reciprocal broadcast.
  - merged matmuls: the two per-core samples share one qk matmul
    (rhs [P, i, s, 256], the ky window is contiguous in the flat (y,t)
    dim) and the two v-conv output halves share one matmul (rhs
    [P, i, co512]). Halves MATMUL+LDWEIGHTS instruction count and
    widens the A^T drain from 256 to 512 columns (fewer, cheaper DVE
    ops). PSUM banks hold one winograd plane for both samples/halves.
  - ONE PSUM tile pool (8 one-bank bufs, single tag) spans conv and
    attention: pool release/realloc at the phase boundary is a full
    barrier (released-zone alloc deps) that idled the PE ~8us and
    re-throttled HAM to K=4/8 for the attention phase.
  - the v-conv units are WOVEN between scores chunks: scores are paced
    by the ScalarE exp ACTIVATEs (~1.25us per chunk vs 0.97us of MM),
    so pure scores blocks stall the PE; v matmuls fill those slots.
  - initial DMAs: the gpsimd software queue sustains ~200+ GB/s, the
    sync/scalar hardware queues only ~40 GB/s. The conv-critical
    stream (u0 per-v chunks + v_in tiles in consumption order, then
    u1..u7) rides gpsimd; the slow queues prefetch the v-conv weights
    (not needed until ~70us in) and later carry the output tiles.
  - drain staging copies split ScalarE (c1) / DVE (c3).
"""

import numpy as np
import ml_dtypes

import concourse.bass as bass
import concourse.tile as tile
from concourse import bacc, mybir
from concourse.bass_utils import run_bass_kernel_spmd

P = 128
B, C, H, W = 16, 512, 32, 32
NCORES = 8
S = B // NCORES
HP = WP = H + 2
NPIX = H * W
CC = C // P
QKCH = 2 * CC
NT = 2
NTILE = 512
TX = W // 4          # 8 x-tiles
NP6 = 6              # winograd positions
KY = 3
NQK = H * TX         # 256: per-sample N for q/k GEMMs
VTC = 2              # v-conv t-chunks per sample (256 t / 128)
SU = 32.0            # q/k weight scale into fp8
SWV = float(2 ** 26)  # folded v*proj weight scale into fp8
VEMIT = 1.0 / 64     # v-conv PSUM -> fp8 emit scale
RSC = 2.0 ** -20     # net 1/(SWV*VEMIT) rides the reciprocal broadcast

BF16 = mybir.dt.bfloat16
F32 = mybir.dt.float32
F8 = mybir.dt.float8e4
EXP = mybir.ActivationFunctionType.Exp
DR = mybir.MatmulPerfMode.DoubleRow
MULT = mybir.AluOpType.mult
ADD = mybir.AluOpType.add

TRACE = False
LAST_EXEC_NS = None

_CACHED = {}

UCH = NP6 * KY * 2 * 2 * P  # bytes/partition of one q/k weight tile
RFL = HP * TX               # flat (y,t) extent per (i,s)


def build_nc(with_bias=True):
    nc = bacc.Bacc()
    vw_d = nc.declare_dram_parameter("vw", [NP6, 2, P, 2 * S * HP * TX], F8,
                                     isOutput=False)
    uw_d = nc.declare_dram_parameter("uw", [QKCH, P, UCH], F8, isOutput=False)
    uv_d = nc.declare_dram_parameter("uv", [P, NP6 * KY * 2 * 2 * CC * P], F8,
                                     isOutput=False)
    bqkv_d = nc.declare_dram_parameter("bqkv", [QKCH, P], BF16, isOutput=False)
    out_d = nc.declare_dram_parameter("out", [S, P, CC, NPIX], BF16, isOutput=True)

    with tile.TileContext(nc) as tc:
        with (
            tc.tile_pool(name="const", bufs=1) as constp,
            tc.tile_pool(name="resid", bufs=1) as resid,
            tc.tile_pool(name="stage", bufs=2) as stage,
        ):
            ones8 = constp.tile([P, 2, 16], F8, name="ones8")
            nc.vector.memset(ones8, 1.0)
            ones_nt = constp.tile([1, NTILE], BF16, name="ones_nt")
            nc.vector.memset(ones_nt, 1.0)
            ones_colT = constp.tile([1, P], BF16, name="ones_colT")
            nc.vector.memset(ones_colT, RSC)

            v_in = {}
            v_qk = {}   # [p, i, s, flat(y,t)] for merged-sample qk matmuls
            v_fl = {}
            for v in range(NP6):
                for j in range(2):
                    t = resid.tile([P, 2 * S * RFL], F8, tag="vin",
                                   bufs=NP6 * 2, name=f"vin_{v}_{j}")
                    v_in[(v, j)] = t
                    v_qk[(v, j)] = t.rearrange("p (i s r) -> p i s r",
                                               i=2, s=S)
                    v_fl[(v, j)] = t.rearrange("p (i r) -> p i r", i=2)

            bias_sb = constp.tile([1, QKCH, P], BF16, name="bias_sb")
            if with_bias:
                nc.sync.dma_start(bias_sb, bqkv_d[:])

            qk8 = {}
            for s in range(S):
                for w8 in ("q", "k"):
                    for j in range(2):
                        qk8[(s, w8, j)] = resid.tile(
                            [P, 2, NPIX], F8, tag="qk8", bufs=S * 4,
                            name=f"{w8}8_{s}_{j}")
            # vT pair tiles: key (s, tc, jp); [t-part, jj%2, co],
            # pixel m = 512*tc + 4p + (2*jp + jj%2); co = folded proj-out ch
            vT8_sb = {}
            for s in range(S):
                for tcn in range(VTC):
                    for jp in range(2):
                        vT8_sb[(s, tcn, jp)] = resid.tile(
                            [P, 2, C], F8, tag="vt", bufs=S * VTC * 2,
                            name=f"vt8_{s}_{tcn}_{jp}")

            uvpool = tc.alloc_tile_pool(name="uv", bufs=1)
            uv_sb = uvpool.tile([P, NP6 * KY * 2 * 2 * CC * P], F8, tag="uv",
                                bufs=1, name="uv")
            uv_vw = uv_sb.rearrange("p (slot i co) -> p slot i co",
                                    slot=NP6 * KY * 2, i=2, co=C)

            upool = tc.alloc_tile_pool(name="uw", bufs=1)
            u_sb = {}
            u_vw = {}
            for oc in range(QKCH):
                t = upool.tile([P, UCH], F8, tag="u", bufs=4, name=f"u_{oc}")
                u_sb[oc] = t
                u_vw[oc] = t.rearrange("p (slot i co) -> p slot i co",
                                       slot=NP6 * KY * 2, i=2, co=P)

            UVC = KY * 2 * 2 * P   # bytes/partition of one v-position of uw
            UVV = KY * 2 * 2 * C   # bytes/partition of one v-position of uv
            nc.gpsimd.dma_start(u_sb[0][:, 0:UVC], uw_d[0][:, 0:UVC])
            nc.gpsimd.dma_start(v_in[(0, 0)], vw_d[0, 0])
            nc.gpsimd.dma_start(v_in[(0, 1)], vw_d[0, 1])
            nc.gpsimd.dma_start(u_sb[0][:, UVC:2 * UVC], uw_d[0][:, UVC:2 * UVC])
            for v in range(1, NP6):
                for j in range(2):
                    nc.gpsimd.dma_start(v_in[(v, j)], vw_d[v, j])
                if v < NP6 - 1:
                    nc.gpsimd.dma_start(
                        u_sb[0][:, (v + 1) * UVC:(v + 2) * UVC],
                        uw_d[0][:, (v + 1) * UVC:(v + 2) * UVC])
            for v in range(2, 4):
                nc.scalar.dma_start(uv_sb[:, v * UVV:(v + 1) * UVV],
                                    uv_d[:, v * UVV:(v + 1) * UVV])
            for v in range(4, NP6):
                nc.sync.dma_start(uv_sb[:, v * UVV:(v + 1) * UVV],
                                  uv_d[:, v * UVV:(v + 1) * UVV])
            for oc in (1, 2):
                nc.gpsimd.dma_start(u_sb[oc], uw_d[oc])
            for v in range(2):
                nc.gpsimd.dma_start(uv_sb[:, v * UVV:(v + 1) * UVV],
                                    uv_d[:, v * UVV:(v + 1) * UVV])
            for oc in range(3, QKCH):
                nc.gpsimd.dma_start(u_sb[oc], uw_d[oc])

            def drain6(pl, emit):
                # pl[v] = [P, 512] PSUM views of the six A^T input planes;
                # rows {1,1,1,1,1,0; 0,1,-1,2,-2,0; 0,1,1,4,4,0;
                # 0,1,-1,8,-8,1}; ScalarE stages M1, DVE stages M3 (one
                # PSUM operand per DVE op); emit(j, o_bf16[P,512])
                n_ = NTILE
                c1 = stage.tile([P, n_], F32, tag="c1", name=f"c1_{drain6.n}")
                c3 = stage.tile([P, n_], F32, tag="c3", name=f"c3_{drain6.n}")
                sp = stage.tile([P, n_], F32, tag="sp", name=f"sp_{drain6.n}")
                sm = stage.tile([P, n_], F32, tag="sm", name=f"sm_{drain6.n}")
                p34 = stage.tile([P, n_], F32, tag="p34", name=f"p34_{drain6.n}")
                m34 = stage.tile([P, n_], F32, tag="m34", name=f"m34_{drain6.n}")
                t0 = stage.tile([P, n_], F32, tag="t0", name=f"t0_{drain6.n}")
                t3 = stage.tile([P, n_], F32, tag="t3", name=f"t3_{drain6.n}")
                o = [stage.tile([P, n_], BF16, tag=f"o{j}",
                                name=f"o{j}_{drain6.n}") for j in range(4)]
                drain6.n += 1
                nc.scalar.copy(out=c1, in_=pl[1])
                nc.vector.tensor_copy(out=c3, in_=pl[3])
                nc.vector.tensor_add(p34, c3, pl[4])
                nc.vector.tensor_add(sp, c1, pl[2])
                nc.vector.tensor_add(t0, pl[0], sp)
                nc.vector.tensor_add(o[0], t0, p34)
                emit(0, o[0])
                nc.vector.tensor_sub(sm, c1, pl[2])
                nc.vector.tensor_sub(m34, c3, pl[4])
                nc.vector.scalar_tensor_tensor(o[1], m34, 2.0, sm,
                                               op0=MULT, op1=ADD)
                emit(1, o[1])
                nc.vector.scalar_tensor_tensor(o[2], p34, 4.0, sp,
                                               op0=MULT, op1=ADD)
                emit(2, o[2])
                nc.vector.scalar_tensor_tensor(t3, m34, 8.0, pl[5],
                                               op0=MULT, op1=ADD)
                nc.vector.tensor_add(o[3], t3, sm)
                emit(3, o[3])
            drain6.n = 0

            # ONE PSUM pool for the whole kernel: 8 rotating 1-bank tiles.
            mpool = tc.alloc_tile_pool(name="mconv", bufs=1, space="PSUM")

            def bank(name):
                return mpool.tile([P, 2, 256], F32, tag="m", bufs=8, name=name)

            # q/k conv: plane v in one bank [P, s(2), 256], both samples per
            # matmul (the ky window is contiguous in the flat (y,t) dim)
            def qk_group(oc):
                pls = [bank(f"m_{oc}_{v}") for v in range(NP6)]
                plv = [t.rearrange("p a n -> p (a n)") for t in pls]
                if with_bias:
                    nc.tensor.matmul(plv[1], lhsT=bias_sb[0:1, oc],
                                     rhs=ones_nt[:, 0:NTILE],
                                     start=True, stop=False)
                for v in range(NP6):
                    for ky in range(KY):
                        for j in range(2):
                            nc.tensor.matmul(
                                pls[v],
                                lhsT=u_vw[oc][:, (v * KY + ky) * 2 + j],
                                rhs=v_qk[(v, j)][:, :, :,
                                                 ky * TX:ky * TX + H * TX],
                                start=((v != 1 or not with_bias)
                                       and ky == 0 and j == 0),
                                stop=(ky == KY - 1 and j == 1),
                                perf_mode=DR)
                if oc < CC:
                    w8, ii = "q", oc
                else:
                    w8, ii = "k", oc - CC

                def emit_qk(j, o, w8=w8, ii=ii):
                    ov = o.rearrange("p (s y tx) -> p s y tx", s=S, tx=TX)
                    for s in range(S):
                        dvv = qk8[(s, w8, ii // 2)].rearrange(
                            "p a (y tx four) -> p a y tx four", four=4, tx=TX)
                        nc.scalar.copy(dvv[:, ii % 2, :, :, j], ov[:, s])
                drain6(plv, emit_qk)

            # v conv: operands swapped -> M = [t(128), co]; both co halves
            # share one matmul (rhs [P, i, co512]); weights carry the folded
            # 1x1 proj. Generator yields after each (v, ky) unit so scores
            # chunks can be woven between units.
            def v_units():
                for s, tcn in [(s, t) for s in range(S) for t in range(VTC)]:
                    pls = [bank(f"mv_{s}_{tcn}_{v}") for v in range(NP6)]
                    plv = [t.rearrange("p a n -> p (a n)") for t in pls]
                    for v in range(NP6):
                        for ky in range(KY):
                            off = s * RFL + (16 * tcn + ky) * TX
                            for j in range(2):
                                nc.tensor.matmul(
                                    plv[v],
                                    lhsT=v_fl[(v, j)][:, :, off:off + P],
                                    rhs=uv_vw[:, (v * KY + ky) * 2 + j],
                                    start=(ky == 0 and j == 0),
                                    stop=(ky == KY - 1 and j == 1),
                                    perf_mode=DR)
                            yield v

                    def emit_v(j, o, s=s, tcn=tcn):
                        nc.scalar.mul(vT8_sb[(s, tcn, j // 2)][:, j % 2, :],
                                      o, VEMIT)
                    drain6(plv, emit_v)

            for oc in range(QKCH):
                qk_group(oc)

            upool.release()

            # ---- attention (reuses the released u-weight SBUF zone; the
            # zone dep is on the last qk LDWEIGHTS, already behind us) ----
            attn = tc.alloc_tile_pool(name="attn", bufs=1)
            exps8 = {}
            for s in range(S):
                for tcn in range(VTC):
                    for jp in range(2):
                        exps8[(s, tcn, jp)] = attn.tile(
                            [P, 2, NPIX], F8, tag="exps",
                            bufs=S * VTC * 2, name=f"exps_{s}_{tcn}_{jp}")
            r_sb = {}
            r_bf = {}
            rbc = {}
            for s in range(S):
                r_sb[s] = attn.tile([1, NPIX], F32, tag="r", bufs=S,
                                    name=f"r_{s}")
                r_bf[s] = attn.tile([1, NPIX], BF16, tag="rb16", bufs=S,
                                    name=f"rb16_{s}")

            def scores_chunk(s, tcn, jp, j2):
                jj = jp * 2 + j2
                ps_nt = [bank(f"ps_sc_{s}_{tcn}_{jj}_{nt}") for nt in range(NT)]
                for j in range(2):
                    kv = qk8[(s, "k", j)].rearrange(
                        "p a (m four) -> p a m four", four=4)
                    lhsT = kv[:, :, P * tcn:P * (tcn + 1), jj]
                    for nt in range(NT):
                        nc.tensor.matmul(
                            ps_nt[nt].rearrange("p a n -> p (a n)"),
                            lhsT=lhsT,
                            rhs=qk8[(s, "q", j)][
                                :, :, nt * NTILE:(nt + 1) * NTILE],
                            start=(j == 0), stop=(j == 1),
                            perf_mode=DR)
                for nt in range(NT):
                    nc.scalar.activation(
                        exps8[(s, tcn, jp)][:, j2, nt * NTILE:(nt + 1) * NTILE],
                        ps_nt[nt].rearrange("p a n -> p (a n)"),
                        EXP, scale=float(C) ** -0.5 / (SU * SU))

            def sums_block(s):
                keys = [(tcn, jp) for tcn in range(VTC) for jp in range(2)]
                for nt in range(NT):
                    ps_sum = bank(f"ps_sum_{s}_{nt}")
                    row = ps_sum.rearrange("p a n -> p (a n)")[0:1, :]
                    for ki, (tcn, jp) in enumerate(keys):
                        nc.tensor.matmul(
                            row, lhsT=ones8[:, :, 0:1],
                            rhs=exps8[(s, tcn, jp)][:, :,
                                                    nt * NTILE:(nt + 1) * NTILE],
                            start=(ki == 0), stop=(ki == len(keys) - 1),
                            perf_mode=DR)
                    nc.vector.reciprocal_approx_fast(
                        out=r_sb[s][:, nt * NTILE:(nt + 1) * NTILE], in_=row)
                nc.scalar.copy(out=r_bf[s], in_=r_sb[s])

            def rbc_block(s):
                for nt in range(NT):
                    ps_b = bank(f"ps_rb_{s}_{nt}")
                    pv = ps_b.rearrange("p a n -> p (a n)")
                    nc.tensor.matmul(pv, lhsT=ones_colT,
                                     rhs=r_bf[s][:, nt * NTILE:(nt + 1) * NTILE],
                                     start=True, stop=True)
                    rb = attn.tile([P, NTILE], F32, tag="rbc", bufs=2 * S,
                                   name=f"rbc_{s}_{nt}")
                    nc.scalar.copy(out=rb, in_=ps_b)
                    rbc[(s, nt)] = rb

            def out_block(s, och):
                # attn@v with proj-folded v: emits final output channels
                keys = [(tcn, jp) for tcn in range(VTC) for jp in range(2)]
                ps_h = [bank(f"ps_h_{s}_{och}_{nt}") for nt in range(NT)]
                for ki, (tcn, jp) in enumerate(keys):
                    lhsT = vT8_sb[(s, tcn, jp)][:, :, och * P:(och + 1) * P]
                    for nt in range(NT):
                        nc.tensor.matmul(
                            ps_h[nt].rearrange("p a n -> p (a n)"), lhsT=lhsT,
                            rhs=exps8[(s, tcn, jp)][
                                :, :, nt * NTILE:(nt + 1) * NTILE],
                            start=(ki == 0), stop=(ki == len(keys) - 1),
                            perf_mode=DR)
                for nt in range(NT):
                    ot = attn.tile([P, NTILE], BF16, tag="ost", bufs=4,
                                   name=f"ot_{s}_{och}_{nt}")
                    nc.vector.tensor_mul(
                        out=ot, in0=ps_h[nt].rearrange("p a n -> p (a n)"),
                        in1=rbc[(s, nt)])
                    eng = nc.sync if (och + nt) % 2 == 0 else nc.scalar
                    eng.dma_start(
                        out_d[s, :, och, nt * NTILE:(nt + 1) * NTILE], ot)

            # weave: the 72 v-conv (v,ky) units fill the PE while the
            # ScalarE exp ACTIVATEs pace the 16 scores chunks
            vu = v_units()

            def take(n):
                for _ in range(n):
                    if next(vu, None) is None:
                        break

            sc_keys = [(s, tcn, jp, j2) for s in range(S) for tcn in range(VTC)
                       for jp in range(2) for j2 in range(2)]
            for c, (s, tcn, jp, j2) in enumerate(sc_keys):
                take(5 if c % 2 else 4)
                if c == 15:
                    take(100)  # flush the last v-group drain ahead of sc15
                scores_chunk(s, tcn, jp, j2)
                if c == 8:
                    sums_block(0)
                if c == 10:
                    rbc_block(0)
                if 12 <= c < 15:
                    out_block(0, c - 12)
            out_block(0, 3)
            sums_block(1)
            rbc_block(1)

            # s=1 outputs: accumulate ki-major across all 8 banks so the
            # first two key rounds (vT from v(1,0)) stream while the last
            # v-group's drain finishes producing vT(1,1)
            keys = [(tcn, jp) for tcn in range(VTC) for jp in range(2)]
            ps_o1 = {(och, nt): bank(f"ps_o1_{och}_{nt}")
                     for och in range(CC) for nt in range(NT)}
            for ki, (tcn, jp) in enumerate(keys):
                for och in range(CC):
                    lhsT = vT8_sb[(1, tcn, jp)][:, :, och * P:(och + 1) * P]
                    for nt in range(NT):
                        nc.tensor.matmul(
                            ps_o1[(och, nt)].rearrange("p a n -> p (a n)"),
                            lhsT=lhsT,
                            rhs=exps8[(1, tcn, jp)][
                                :, :, nt * NTILE:(nt + 1) * NTILE],
                            start=(ki == 0), stop=(ki == len(keys) - 1),
                            perf_mode=DR)
            for och in range(CC):
                for nt in range(NT):
                    ot = attn.tile([P, NTILE], BF16, tag="ost", bufs=4,
                                   name=f"ot1_{och}_{nt}")
                    nc.vector.tensor_mul(
                        out=ot,
                        in0=ps_o1[(och, nt)].rearrange("p a n -> p (a n)"),
                        in1=rbc[(1, nt)])
                    eng = nc.sync if (och + nt) % 2 == 0 else nc.scalar
                    eng.dma_start(
                        out_d[1, :, och, nt * NTILE:(nt + 1) * NTILE], ot)

            mpool.release()
            attn.release()
            uvpool.release()

    nc.finalize()
    return nc


BT43 = np.array([
    [4, 0, -5, 0, 1, 0],
    [0, -4, -4, 1, 1, 0],
    [0, 4, -4, -1, 1, 0],
    [0, -2, -1, 2, 1, 0],
    [0, 2, -1, -2, 1, 0],
    [0, 4, 0, -5, 0, 1]], np.float32)
G43 = np.array([
    [1 / 4, 0, 0],
    [-1 / 6, -1 / 6, -1 / 6],
    [-1 / 6, 1 / 6, -1 / 6],
    [1 / 24, 1 / 12, 1 / 6],
    [1 / 24, -1 / 12, 1 / 6],
    [0, 0, 1]], np.float32)


def prep_inputs(x, w_qkv, b_qkv, w_proj):
    e4 = ml_dtypes.float8_e4m3
    xpad = np.zeros((B, C, HP, WP), np.float32)
    xpad[:, :, 1:H + 1, 1:W + 1] = x

    taps = np.stack([xpad[:, :, :, a:a + 4 * TX:4][:, :, :, :TX]
                     for a in range(6)])          # [6, B, C, HP, TX]
    V = np.tensordot(BT43, taps, axes=([1], [0]))  # [6, B, C, HP, TX]
    vw = np.ascontiguousarray(
        V.reshape(NP6, B, 2, 2, P, HP, TX)
        .transpose(0, 2, 4, 3, 1, 5, 6)).astype(e4)  # [6, 2, P, 2, B, HP, TX]

    u6qk = np.tensordot(w_qkv[:2 * C] * SU, G43, axes=([3], [1]))
    uw = np.ascontiguousarray(
        u6qk.reshape(QKCH, P, 2, 2, P, KY, NP6)
        .transpose(0, 4, 6, 5, 2, 3, 1)
        .reshape(QKCH, P, UCH)).astype(e4)
    # fold the 1x1 proj into the v weights: conv(x, Wp @ wv) == proj(conv(x, wv))
    u6v = np.tensordot(w_qkv[2 * C:], G43, axes=([3], [1]))  # [vc, ci, ky, 6]
    u6vp = np.tensordot(w_proj[:, :, 0, 0], u6v, axes=([1], [0])) * SWV
    uv = np.ascontiguousarray(
        u6vp.reshape(C, 2, 2, P, KY, NP6)
        .transpose(3, 5, 4, 1, 2, 0)
        .reshape(P, NP6 * KY * 2 * 2 * C)).astype(e4)
    bqkv = np.ascontiguousarray((b_qkv[:2 * C] * SU).reshape(QKCH, P)).astype(
        ml_dtypes.bfloat16)
    return vw, uw, uv, bqkv


def kernel(x, w_qkv, b_qkv, w_proj, b_proj, gn_gamma=None, gn_beta=None):
    global LAST_EXEC_NS
    x = np.asarray(x, np.float32)
    w_qkv = np.asarray(w_qkv, np.float32)
    b_qkv = np.asarray(b_qkv, np.float32)
    w_proj = np.asarray(w_proj, np.float32)
    b_proj = np.asarray(b_proj, np.float32)

    with_bias = bool(np.any(b_qkv[:2 * C]))
    if with_bias not in _CACHED:
        _CACHED[with_bias] = build_nc(with_bias=with_bias)
    nc = _CACHED[with_bias]

    vw, uw, uv, bqkv = prep_inputs(x, w_qkv, b_qkv, w_proj)

    in_maps = []
    for core in range(NCORES):
        sl = slice(core * S, (core + 1) * S)
        in_maps.append({
            "vw": np.ascontiguousarray(vw[:, :, :, :, sl]).reshape(NP6, 2, P, -1),
            "uw": uw,
            "uv": uv,
            "bqkv": bqkv,
        })

    res = run_bass_kernel_spmd(nc, in_maps, list(range(NCORES)), trace=TRACE)
    LAST_EXEC_NS = res.exec_time_ns
    h = np.stack([np.asarray(res.results[c]["out"], np.float32)
                  for c in range(NCORES)])
    h = h.reshape(B, P, CC, NPIX).transpose(0, 2, 1, 3).reshape(B, C, H, W)
    out = x + h + b_proj[None, :, None, None]
    # v-bias passes through softmax-weighting as a constant channel offset
    bv = b_qkv[2 * C:]
    if np.any(bv):
        out = out + (w_proj[:, :, 0, 0] @ bv)[None, :, None, None]
    return np.ascontiguousarray(out).astype(np.float32, copy=False)


# revision 15
# speedup vs baseline: 1.0492x; 1.0402x over previous
"""AttnBlock on 8 TRN2 NeuronCores -- F(4,3) x-Winograd, folded-proj variant.

Same conv math as the F(4,3) baseline (host-side input/weight transforms,
fp8 DoubleRow GEMMs with fp32 PSUM, v produced pre-transposed by swapping
conv operands, deferred softmax normalization), restructured for overlap:

  - the 1x1 proj conv is folded into the v-conv weights on the host
    (both are linear maps over channels), so attn@v emits the final
    output channels directly: no hN intermediate, no proj GEMM stage.
    The tiny w_proj magnitudes (~1e-7) are scaled up by 2^26 into fp8
    range; the 2^-20 net factor rides the reciprocal broadcast.
  - merged matmuls: the two per-core samples share one qk matmul
    (rhs [P, i, s, 256], the ky window is contiguous in the flat (y,t)
    dim) and the two v-conv output halves share one matmul (rhs
    [P, i, co512]). Halves MATMUL+LDWEIGHTS instruction count and
    widens the A^T drain from 256 to 512 columns (fewer, cheaper DVE
    ops). PSUM banks hold one winograd plane for both samples/halves.
  - ONE PSUM tile pool (8 one-bank bufs, single tag) spans conv and
    attention: pool release/realloc at the phase boundary is a full
    barrier (released-zone alloc deps) that idled the PE ~8us and
    re-throttled HAM to K=4/8 for the attention phase.
  - the v-conv units are WOVEN between scores chunks: scores are paced
    by the ScalarE exp ACTIVATEs (~1.25us per chunk vs 0.97us of MM),
    so pure scores blocks stall the PE; v matmuls fill those slots.
  - initial DMAs: the gpsimd software queue sustains ~200+ GB/s, the
    sync/scalar hardware queues only ~40 GB/s. The conv-critical
    stream (u0 per-v chunks + v_in tiles in consumption order, then
    u1..u7) rides gpsimd; the slow queues prefetch the v-conv weights
    (not needed until ~70us in) and later carry the output tiles.
  - drain staging copies split ScalarE (c1) / DVE (c3).
"""

import numpy as np
import ml_dtypes

import concourse.bass as bass
import concourse.tile as tile
from concourse import bacc, mybir
from concourse.bass_utils import run_bass_kernel_spmd

P = 128
B, C, H, W = 16, 512, 32, 32
NCORES = 8
S = B // NCORES
HP = WP = H + 2
NPIX = H * W
CC = C // P
QKCH = 2 * CC
NT = 2
NTILE = 512
TX = W // 4          # 8 x-tiles
NP6 = 6              # winograd positions
KY = 3
NQK = H * TX         # 256: per-sample N for q/k GEMMs
VTC = 2              # v-conv t-chunks per sample (256 t / 128)
SU = 32.0            # q/k weight scale into fp8
SWV = float(2 ** 26)  # folded v*proj weight scale into fp8
VEMIT = 1.0 / 64     # v-conv PSUM -> fp8 emit scale
RSC = 2.0 ** -20     # net 1/(SWV*VEMIT) rides the reciprocal broadcast

BF16 = mybir.dt.bfloat16
F32 = mybir.dt.float32
F8 = mybir.dt.float8e4
EXP = mybir.ActivationFunctionType.Exp
DR = mybir.MatmulPerfMode.DoubleRow
MULT = mybir.AluOpType.mult
ADD = mybir.AluOpType.add

TRACE = False
LAST_EXEC_NS = None

_CACHED = {}

UCH = NP6 * KY * 2 * 2 * P  # bytes/partition of one q/k weight tile
RFL = HP * TX               # flat (y,t) extent per (i,s)


def build_nc(with_bias=True):
    nc = bacc.Bacc()
    vw_d = nc.declare_dram_parameter("vw", [NP6, 2, P, 2 * S * HP * TX], F8,
                                     isOutput=False)
    uw_d = nc.declare_dram_parameter("uw", [QKCH, P, UCH], F8, isOutput=False)
    uv_d = nc.declare_dram_parameter("uv", [P, NP6 * KY * 2 * 2 * CC * P], F8,
                                     isOutput=False)
    bqkv_d = nc.declare_dram_parameter("bqkv", [QKCH, P], BF16, isOutput=False)
    out_d = nc.declare_dram_parameter("out", [S, P, CC, NPIX], BF16, isOutput=True)

    with tile.TileContext(nc) as tc:
        with (
            tc.tile_pool(name="const", bufs=1) as constp,
            tc.tile_pool(name="resid", bufs=1) as resid,
            tc.tile_pool(name="stage", bufs=2) as stage,
        ):
            ones8 = constp.tile([P, 2, 16], F8, name="ones8")
            nc.vector.memset(ones8, 1.0)
            ones_nt = constp.tile([1, NTILE], BF16, name="ones_nt")
            nc.vector.memset(ones_nt, 1.0)

            v_in = {}
            v_qk = {}   # [p, i, s, flat(y,t)] for merged-sample qk matmuls
            v_fl = {}
            for v in range(NP6):
                for j in range(2):
                    t = resid.tile([P, 2 * S * RFL], F8, tag="vin",
                                   bufs=NP6 * 2, name=f"vin_{v}_{j}")
                    v_in[(v, j)] = t
                    v_qk[(v, j)] = t.rearrange("p (i s r) -> p i s r",
                                               i=2, s=S)
                    v_fl[(v, j)] = t.rearrange("p (i r) -> p i r", i=2)

            bias_sb = constp.tile([1, QKCH, P], BF16, name="bias_sb")
            if with_bias:
                nc.sync.dma_start(bias_sb, bqkv_d[:])

            qk8 = {}
            for s in range(S):
                for w8 in ("q", "k"):
                    for j in range(2):
                        qk8[(s, w8, j)] = resid.tile(
                            [P, 2, NPIX], F8, tag="qk8", bufs=S * 4,
                            name=f"{w8}8_{s}_{j}")
            # vT pair tiles: key (s, tc, jp); [t-part, jj%2, co],
            # pixel m = 512*tc + 4p + (2*jp + jj%2); co = folded proj-out ch
            vT8_sb = {}
            for s in range(S):
                for tcn in range(VTC):
                    for jp in range(2):
                        vT8_sb[(s, tcn, jp)] = resid.tile(
                            [P, 2, C], F8, tag="vt", bufs=S * VTC * 2,
                            name=f"vt8_{s}_{tcn}_{jp}")

            uvpool = tc.alloc_tile_pool(name="uv", bufs=1)
            uv_sb = uvpool.tile([P, NP6 * KY * 2 * 2 * CC * P], F8, tag="uv",
                                bufs=1, name="uv")
            uv_vw = uv_sb.rearrange("p (slot i co) -> p slot i co",
                                    slot=NP6 * KY * 2, i=2, co=C)

            upool = tc.alloc_tile_pool(name="uw", bufs=1)
            u_sb = {}
            u_vw = {}
            for oc in range(QKCH):
                t = upool.tile([P, UCH], F8, tag="u", bufs=4, name=f"u_{oc}")
                u_sb[oc] = t
                u_vw[oc] = t.rearrange("p (slot i co) -> p slot i co",
                                       slot=NP6 * KY * 2, i=2, co=P)

            UVC = KY * 2 * 2 * P   # bytes/partition of one v-position of uw
            UVV = KY * 2 * 2 * C   # bytes/partition of one v-position of uv
            nc.sync.dma_start(v_in[(0, 0)], vw_d[0, 0])
            nc.sync.dma_start(v_in[(0, 1)], vw_d[0, 1])
            nc.gpsimd.dma_start(u_sb[0][:, 0:UVC], uw_d[0][:, 0:UVC])
            nc.gpsimd.dma_start(u_sb[0][:, UVC:2 * UVC], uw_d[0][:, UVC:2 * UVC])
            for v in range(1, NP6):
                for j in range(2):
                    nc.gpsimd.dma_start(v_in[(v, j)], vw_d[v, j])
                if v < NP6 - 1:
                    nc.gpsimd.dma_start(
                        u_sb[0][:, (v + 1) * UVC:(v + 2) * UVC],
                        uw_d[0][:, (v + 1) * UVC:(v + 2) * UVC])
            for v in range(2, 4):
                nc.scalar.dma_start(uv_sb[:, v * UVV:(v + 1) * UVV],
                                    uv_d[:, v * UVV:(v + 1) * UVV])
            for v in range(4, NP6):
                nc.sync.dma_start(uv_sb[:, v * UVV:(v + 1) * UVV],
                                  uv_d[:, v * UVV:(v + 1) * UVV])
            for oc in (1, 2):
                nc.gpsimd.dma_start(u_sb[oc], uw_d[oc])
            for v in range(2):
                nc.gpsimd.dma_start(uv_sb[:, v * UVV:(v + 1) * UVV],
                                    uv_d[:, v * UVV:(v + 1) * UVV])
            for oc in range(3, QKCH):
                nc.gpsimd.dma_start(u_sb[oc], uw_d[oc])

            def drain6(pl, emit):
                # pl[v] = [P, 512] PSUM views of the six A^T input planes;
                # rows {1,1,1,1,1,0; 0,1,-1,2,-2,0; 0,1,1,4,4,0;
                # 0,1,-1,8,-8,1}; ScalarE stages M1, DVE stages M3 (one
                # PSUM operand per DVE op); emit(j, o_bf16[P,512])
                n_ = NTILE
                c1 = stage.tile([P, n_], F32, tag="c1", name=f"c1_{drain6.n}")
                c3 = stage.tile([P, n_], F32, tag="c3", name=f"c3_{drain6.n}")
                sp = stage.tile([P, n_], F32, tag="sp", name=f"sp_{drain6.n}")
                sm = stage.tile([P, n_], F32, tag="sm", name=f"sm_{drain6.n}")
                p34 = stage.tile([P, n_], F32, tag="p34", name=f"p34_{drain6.n}")
                m34 = stage.tile([P, n_], F32, tag="m34", name=f"m34_{drain6.n}")
                t0 = stage.tile([P, n_], F32, tag="t0", name=f"t0_{drain6.n}")
                t3 = stage.tile([P, n_], F32, tag="t3", name=f"t3_{drain6.n}")
                o = [stage.tile([P, n_], BF16, tag=f"o{j}",
                                name=f"o{j}_{drain6.n}") for j in range(4)]
                drain6.n += 1
                nc.scalar.copy(out=c1, in_=pl[1])
                nc.vector.tensor_copy(out=c3, in_=pl[3])
                nc.vector.tensor_add(p34, c3, pl[4])
                nc.vector.tensor_add(sp, c1, pl[2])
                nc.vector.tensor_add(t0, pl[0], sp)
                nc.vector.tensor_add(o[0], t0, p34)
                emit(0, o[0])
                nc.vector.tensor_sub(sm, c1, pl[2])
                nc.vector.tensor_sub(m34, c3, pl[4])
                nc.vector.scalar_tensor_tensor(o[1], m34, 2.0, sm,
                                               op0=MULT, op1=ADD)
                emit(1, o[1])
                nc.vector.scalar_tensor_tensor(o[2], p34, 4.0, sp,
                                               op0=MULT, op1=ADD)
                emit(2, o[2])
                nc.vector.scalar_tensor_tensor(t3, m34, 8.0, pl[5],
                                               op0=MULT, op1=ADD)
                nc.vector.tensor_add(o[3], t3, sm)
                emit(3, o[3])
            drain6.n = 0

            # ONE PSUM pool for the whole kernel: 8 rotating 1-bank tiles.
            mpool = tc.alloc_tile_pool(name="mconv", bufs=1, space="PSUM")

            def bank(name):
                return mpool.tile([P, 2, 256], F32, tag="m", bufs=8, name=name)

            # q/k conv: plane v in one bank [P, s(2), 256], both samples per
            # matmul (the ky window is contiguous in the flat (y,t) dim)
            def qk_group(oc):
                pls = [bank(f"m_{oc}_{v}") for v in range(NP6)]
                plv = [t.rearrange("p a n -> p (a n)") for t in pls]
                if with_bias:
                    nc.tensor.matmul(plv[1], lhsT=bias_sb[0:1, oc],
                                     rhs=ones_nt[:, 0:NTILE],
                                     start=True, stop=False)
                for v in range(NP6):
                    for ky in range(KY):
                        for j in range(2):
                            nc.tensor.matmul(
                                pls[v],
                                lhsT=u_vw[oc][:, (v * KY + ky) * 2 + j],
                                rhs=v_qk[(v, j)][:, :, :,
                                                 ky * TX:ky * TX + H * TX],
                                start=((v != 1 or not with_bias)
                                       and ky == 0 and j == 0),
                                stop=(ky == KY - 1 and j == 1),
                                perf_mode=DR)
                if oc < CC:
                    w8, ii = "q", oc
                else:
                    w8, ii = "k", oc - CC

                def emit_qk(j, o, w8=w8, ii=ii):
                    ov = o.rearrange("p (s y tx) -> p s y tx", s=S, tx=TX)
                    for s in range(S):
                        dvv = qk8[(s, w8, ii // 2)].rearrange(
                            "p a (y tx four) -> p a y tx four", four=4, tx=TX)
                        nc.scalar.copy(dvv[:, ii % 2, :, :, j], ov[:, s])
                drain6(plv, emit_qk)

            # v conv: operands swapped -> M = [t(128), co]; both co halves
            # share one matmul (rhs [P, i, co512]); weights carry the folded
            # 1x1 proj. Generator yields after each (v, ky) unit so scores
            # chunks can be woven between units.
            def v_units():
                for s, tcn in [(s, t) for s in range(S) for t in range(VTC)]:
                    pls = [bank(f"mv_{s}_{tcn}_{v}") for v in range(NP6)]
                    plv = [t.rearrange("p a n -> p (a n)") for t in pls]
                    for v in range(NP6):
                        for ky in range(KY):
                            off = s * RFL + (16 * tcn + ky) * TX
                            for j in range(2):
                                nc.tensor.matmul(
                                    plv[v],
                                    lhsT=v_fl[(v, j)][:, :, off:off + P],
                                    rhs=uv_vw[:, (v * KY + ky) * 2 + j],
                                    start=(ky == 0 and j == 0),
                                    stop=(ky == KY - 1 and j == 1),
                                    perf_mode=DR)
                            yield v

                    def emit_v(j, o, s=s, tcn=tcn):
                        nc.scalar.mul(vT8_sb[(s, tcn, j // 2)][:, j % 2, :],
                                      o, VEMIT)
                    drain6(plv, emit_v)

            for oc in range(QKCH):
                qk_group(oc)

            upool.release()

            # ---- attention (reuses the released u-weight SBUF zone; the
            # zone dep is on the last qk LDWEIGHTS, already behind us) ----
            attn = tc.alloc_tile_pool(name="attn", bufs=1)
            exps8 = {}
            for s in range(S):
                for tcn in range(VTC):
                    for jp in range(2):
                        exps8[(s, tcn, jp)] = attn.tile(
                            [P, 2, NPIX], F8, tag="exps",
                            bufs=S * VTC * 2, name=f"exps_{s}_{tcn}_{jp}")
            r_sb = {}
            rb_bc = {}
            for s in range(S):
                r_sb[s] = attn.tile([1, NPIX], F32, tag="r", bufs=S,
                                    name=f"r_{s}")
                rb_bc[s] = attn.tile([P, NPIX], F32, tag="rbcb", bufs=S,
                                     name=f"rbcb_{s}")

            def scores_chunk(s, tcn, jp, j2):
                jj = jp * 2 + j2
                ps_nt = [bank(f"ps_sc_{s}_{tcn}_{jj}_{nt}") for nt in range(NT)]
                for j in range(2):
                    kv = qk8[(s, "k", j)].rearrange(
                        "p a (m four) -> p a m four", four=4)
                    lhsT = kv[:, :, P * tcn:P * (tcn + 1), jj]
                    for nt in range(NT):
                        nc.tensor.matmul(
                            ps_nt[nt].rearrange("p a n -> p (a n)"),
                            lhsT=lhsT,
                            rhs=qk8[(s, "q", j)][
                                :, :, nt * NTILE:(nt + 1) * NTILE],
                            start=(j == 0), stop=(j == 1),
                            perf_mode=DR)
                for nt in range(NT):
                    nc.scalar.activation(
                        exps8[(s, tcn, jp)][:, j2, nt * NTILE:(nt + 1) * NTILE],
                        ps_nt[nt].rearrange("p a n -> p (a n)"),
                        EXP, scale=float(C) ** -0.5 / (SU * SU))

            def sums_block(s):
                keys = [(tcn, jp) for tcn in range(VTC) for jp in range(2)]
                for nt in range(NT):
                    ps_sum = bank(f"ps_sum_{s}_{nt}")
                    row = ps_sum.rearrange("p a n -> p (a n)")[0:1, :]
                    for ki, (tcn, jp) in enumerate(keys):
                        nc.tensor.matmul(
                            row, lhsT=ones8[:, :, 0:1],
                            rhs=exps8[(s, tcn, jp)][:, :,
                                                    nt * NTILE:(nt + 1) * NTILE],
                            start=(ki == 0), stop=(ki == len(keys) - 1),
                            perf_mode=DR)
                    nc.vector.reciprocal_approx_fast(
                        out=r_sb[s][:, nt * NTILE:(nt + 1) * NTILE], in_=row)
                nc.gpsimd.partition_broadcast(rb_bc[s], r_sb[s])

            def out_block(s, och):
                # attn@v with proj-folded v: emits final output channels
                keys = [(tcn, jp) for tcn in range(VTC) for jp in range(2)]
                ps_h = [bank(f"ps_h_{s}_{och}_{nt}") for nt in range(NT)]
                for ki, (tcn, jp) in enumerate(keys):
                    lhsT = vT8_sb[(s, tcn, jp)][:, :, och * P:(och + 1) * P]
                    for nt in range(NT):
                        nc.tensor.matmul(
                            ps_h[nt].rearrange("p a n -> p (a n)"), lhsT=lhsT,
                            rhs=exps8[(s, tcn, jp)][
                                :, :, nt * NTILE:(nt + 1) * NTILE],
                            start=(ki == 0), stop=(ki == len(keys) - 1),
                            perf_mode=DR)
                for nt in range(NT):
                    ot = attn.tile([P, NTILE], BF16, tag="ost", bufs=4,
                                   name=f"ot_{s}_{och}_{nt}")
                    nc.vector.scalar_tensor_tensor(
                        ot, ps_h[nt].rearrange("p a n -> p (a n)"), RSC,
                        rb_bc[s][:, nt * NTILE:(nt + 1) * NTILE],
                        op0=MULT, op1=MULT)
                    eng = nc.sync if (och + nt) % 2 == 0 else nc.scalar
                    eng.dma_start(
                        out_d[s, :, och, nt * NTILE:(nt + 1) * NTILE], ot)

            # weave: the 72 v-conv (v,ky) units fill the PE while the
            # ScalarE exp ACTIVATEs pace the 16 scores chunks
            vu = v_units()

            def take(n):
                for _ in range(n):
                    if next(vu, None) is None:
                        break

            sc_keys = [(s, tcn, jp, j2) for s in range(S) for tcn in range(VTC)
                       for jp in range(2) for j2 in range(2)]
            for c, (s, tcn, jp, j2) in enumerate(sc_keys):
                take(5 if c % 2 else 4)
                if c == 15:
                    take(100)  # flush the last v-group drain ahead of sc15
                scores_chunk(s, tcn, jp, j2)
                if c == 8:
                    sums_block(0)
                if 12 <= c < 15:
                    out_block(0, c - 12)
            out_block(0, 3)
            sums_block(1)

            # s=1 outputs: accumulate ki-major across all 8 banks so the
            # first two key rounds (vT from v(1,0)) stream while the last
            # v-group's drain finishes producing vT(1,1); the reciprocal
            # broadcast for s=1 rides between rounds so its serial chain
            # (sum MMs -> recip -> bf16 copy -> matmul) hides under them
            keys = [(tcn, jp) for tcn in range(VTC) for jp in range(2)]
            ps_o1 = {(och, nt): bank(f"ps_o1_{och}_{nt}")
                     for och in range(CC) for nt in range(NT)}
            for ki, (tcn, jp) in enumerate(keys):
                for och in range(CC):
                    lhsT = vT8_sb[(1, tcn, jp)][:, :, och * P:(och + 1) * P]
                    for nt in range(NT):
                        nc.tensor.matmul(
                            ps_o1[(och, nt)].rearrange("p a n -> p (a n)"),
                            lhsT=lhsT,
                            rhs=exps8[(1, tcn, jp)][
                                :, :, nt * NTILE:(nt + 1) * NTILE],
                            start=(ki == 0), stop=(ki == len(keys) - 1),
                            perf_mode=DR)
            for och in range(CC):
                for nt in range(NT):
                    ot = attn.tile([P, NTILE], BF16, tag="ost", bufs=4,
                                   name=f"ot1_{och}_{nt}")
                    nc.vector.scalar_tensor_tensor(
                        ot, ps_o1[(och, nt)].rearrange("p a n -> p (a n)"),
                        RSC, rb_bc[1][:, nt * NTILE:(nt + 1) * NTILE],
                        op0=MULT, op1=MULT)
                    eng = nc.sync if (och + nt) % 2 == 0 else nc.scalar
                    eng.dma_start(
                        out_d[1, :, och, nt * NTILE:(nt + 1) * NTILE], ot)

            mpool.release()
            attn.release()
            uvpool.release()

    nc.finalize()
    return nc


BT43 = np.array([
    [4, 0, -5, 0, 1, 0],
    [0, -4, -4, 1, 1, 0],
    [0, 4, -4, -1, 1, 0],
    [0, -2, -1, 2, 1, 0],
    [0, 2, -1, -2, 1, 0],
    [0, 4, 0, -5, 0, 1]], np.float32)
G43 = np.array([
    [1 / 4, 0, 0],
    [-1 / 6, -1 / 6, -1 / 6],
    [-1 / 6, 1 / 6, -1 / 6],
    [1 / 24, 1 / 12, 1 / 6],
    [1 / 24, -1 / 12, 1 / 6],
    [0, 0, 1]], np.float32)


def prep_inputs(x, w_qkv, b_qkv, w_proj):
    e4 = ml_dtypes.float8_e4m3
    xpad = np.zeros((B, C, HP, WP), np.float32)
    xpad[:, :, 1:H + 1, 1:W + 1] = x

    taps = np.stack([xpad[:, :, :, a:a + 4 * TX:4][:, :, :, :TX]
                     for a in range(6)])          # [6, B, C, HP, TX]
    V = np.tensordot(BT43, taps, axes=([1], [0]))  # [6, B, C, HP, TX]
    vw = np.ascontiguousarray(
        V.reshape(NP6, B, 2, 2, P, HP, TX)
        .transpose(0, 2, 4, 3, 1, 5, 6)).astype(e4)  # [6, 2, P, 2, B, HP, TX]

    u6qk = np.tensordot(w_qkv[:2 * C] * SU, G43, axes=([3], [1]))
    uw = np.ascontiguousarray(
        u6qk.reshape(QKCH, P, 2, 2, P, KY, NP6)
        .transpose(0, 4, 6, 5, 2, 3, 1)
        .reshape(QKCH, P, UCH)).astype(e4)
    # fold the 1x1 proj into the v weights: conv(x, Wp @ wv) == proj(conv(x, wv))
    u6v = np.tensordot(w_qkv[2 * C:], G43, axes=([3], [1]))  # [vc, ci, ky, 6]
    u6vp = np.tensordot(w_proj[:, :, 0, 0], u6v, axes=([1], [0])) * SWV
    uv = np.ascontiguousarray(
        u6vp.reshape(C, 2, 2, P, KY, NP6)
        .transpose(3, 5, 4, 1, 2, 0)
        .reshape(P, NP6 * KY * 2 * 2 * C)).astype(e4)
    bqkv = np.ascontiguousarray((b_qkv[:2 * C] * SU).reshape(QKCH, P)).astype(
        ml_dtypes.bfloat16)
    return vw, uw, uv, bqkv


def kernel(x, w_qkv, b_qkv, w_proj, b_proj, gn_gamma=None, gn_beta=None):
    global LAST_EXEC_NS
    x = np.asarray(x, np.float32)
    w_qkv = np.asarray(w_qkv, np.float32)
    b_qkv = np.asarray(b_qkv, np.float32)
    w_proj = np.asarray(w_proj, np.float32)
    b_proj = np.asarray(b_proj, np.float32)

    with_bias = bool(np.any(b_qkv[:2 * C]))
    if with_bias not in _CACHED:
        _CACHED[with_bias] = build_nc(with_bias=with_bias)
    nc = _CACHED[with_bias]

    vw, uw, uv, bqkv = prep_inputs(x, w_qkv, b_qkv, w_proj)

    in_maps = []
    for core in range(NCORES):
        sl = slice(core * S, (core + 1) * S)
        in_maps.append({
            "vw": np.ascontiguousarray(vw[:, :, :, :, sl]).reshape(NP6, 2, P, -1),
            "uw": uw,
            "uv": uv,
            "bqkv": bqkv,
        })

    res = run_bass_kernel_spmd(nc, in_maps, list(range(NCORES)), trace=TRACE)
    LAST_EXEC_NS = res.exec_time_ns
    h = np.stack([np.asarray(res.results[c]["out"], np.float32)
                  for c in range(NCORES)])
    h = h.reshape(B, P, CC, NPIX).transpose(0, 2, 1, 3).reshape(B, C, H, W)
    out = x + h + b_proj[None, :, None, None]
    # v-bias passes through softmax-weighting as a constant channel offset
    bv = b_qkv[2 * C:]
    if np.any(bv):
        out = out + (w_proj[:, :, 0, 0] @ bv)[None, :, None, None]
    return np.ascontiguousarray(out).astype(np.float32, copy=False)


# revision 16
# speedup vs baseline: 1.0541x; 1.0047x over previous
"""AttnBlock on 8 TRN2 NeuronCores -- F(4,3) x-Winograd, folded-proj variant.

Same conv math as the F(4,3) baseline (host-side input/weight transforms,
fp8 DoubleRow GEMMs with fp32 PSUM, v produced pre-transposed by swapping
conv operands, deferred softmax normalization), restructured for overlap:

  - the 1x1 proj conv is folded into the v-conv weights on the host
    (both are linear maps over channels), so attn@v emits the final
    output channels directly: no hN intermediate, no proj GEMM stage.
    The tiny w_proj magnitudes (~1e-7) are scaled up by 2^26 into fp8
    range; the 2^-20 net factor rides the reciprocal broadcast.
  - merged matmuls: the two per-core samples share one qk matmul
    (rhs [P, i, s, 256], the ky window is contiguous in the flat (y,t)
    dim) and the two v-conv output halves share one matmul (rhs
    [P, i, co512]). Halves MATMUL+LDWEIGHTS instruction count and
    widens the A^T drain from 256 to 512 columns (fewer, cheaper DVE
    ops). PSUM banks hold one winograd plane for both samples/halves.
  - ONE PSUM tile pool (8 one-bank bufs, single tag) spans conv and
    attention: pool release/realloc at the phase boundary is a full
    barrier (released-zone alloc deps) that idled the PE ~8us and
    re-throttled HAM to K=4/8 for the attention phase.
  - the v-conv units are WOVEN between scores chunks: scores are paced
    by the ScalarE exp ACTIVATEs (~1.25us per chunk vs 0.97us of MM),
    so pure scores blocks stall the PE; v matmuls fill those slots.
  - initial DMAs: the gpsimd software queue sustains ~200+ GB/s, the
    sync/scalar hardware queues only ~40 GB/s. The conv-critical
    stream (u0 per-v chunks + v_in tiles in consumption order, then
    u1..u7) rides gpsimd; the slow queues prefetch the v-conv weights
    (not needed until ~70us in) and later carry the output tiles.
  - drain staging copies split ScalarE (c1) / DVE (c3).
"""

import numpy as np
import ml_dtypes

import concourse.bass as bass
import concourse.tile as tile
from concourse import bacc, mybir
from concourse.bass_utils import run_bass_kernel_spmd

P = 128
B, C, H, W = 16, 512, 32, 32
NCORES = 8
S = B // NCORES
HP = WP = H + 2
NPIX = H * W
CC = C // P
QKCH = 2 * CC
NT = 2
NTILE = 512
TX = W // 4          # 8 x-tiles
NP6 = 6              # winograd positions
KY = 3
NQK = H * TX         # 256: per-sample N for q/k GEMMs
VTC = 2              # v-conv t-chunks per sample (256 t / 128)
SU = 32.0            # q/k weight scale into fp8
SWV = float(2 ** 26)  # folded v*proj weight scale into fp8
VEMIT = 1.0 / 64     # v-conv PSUM -> fp8 emit scale
RSC = 2.0 ** -20     # net 1/(SWV*VEMIT) rides the reciprocal broadcast

BF16 = mybir.dt.bfloat16
F32 = mybir.dt.float32
F8 = mybir.dt.float8e4
EXP = mybir.ActivationFunctionType.Exp
DR = mybir.MatmulPerfMode.DoubleRow
MULT = mybir.AluOpType.mult
ADD = mybir.AluOpType.add

TRACE = False
LAST_EXEC_NS = None

_CACHED = {}

UCH = NP6 * KY * 2 * 2 * P  # bytes/partition of one q/k weight tile
RFL = HP * TX               # flat (y,t) extent per (i,s)


def build_nc(with_bias=True):
    nc = bacc.Bacc()
    vw_d = nc.declare_dram_parameter("vw", [NP6, 2, P, 2 * S * HP * TX], F8,
                                     isOutput=False)
    uw_d = nc.declare_dram_parameter("uw", [QKCH, P, UCH], F8, isOutput=False)
    uv_d = nc.declare_dram_parameter("uv", [P, NP6 * KY * 2 * 2 * CC * P], F8,
                                     isOutput=False)
    bqkv_d = nc.declare_dram_parameter("bqkv", [QKCH, P], BF16, isOutput=False)
    out_d = nc.declare_dram_parameter("out", [S, P, CC, NPIX], BF16, isOutput=True)

    with tile.TileContext(nc) as tc:
        with (
            tc.tile_pool(name="const", bufs=1) as constp,
            tc.tile_pool(name="resid", bufs=1) as resid,
            tc.tile_pool(name="stage", bufs=2) as stage,
        ):
            ones8 = constp.tile([P, 2, 16], F8, name="ones8")
            nc.vector.memset(ones8, 1.0)
            ones_nt = constp.tile([1, NTILE], BF16, name="ones_nt")
            nc.vector.memset(ones_nt, 1.0)

            v_in = {}
            v_qk = {}   # [p, i, s, flat(y,t)] for merged-sample qk matmuls
            v_fl = {}
            for v in range(NP6):
                for j in range(2):
                    t = resid.tile([P, 2 * S * RFL], F8, tag="vin",
                                   bufs=NP6 * 2, name=f"vin_{v}_{j}")
                    v_in[(v, j)] = t
                    v_qk[(v, j)] = t.rearrange("p (i s r) -> p i s r",
                                               i=2, s=S)
                    v_fl[(v, j)] = t.rearrange("p (i r) -> p i r", i=2)

            bias_sb = constp.tile([1, QKCH, P], BF16, name="bias_sb")
            if with_bias:
                nc.sync.dma_start(bias_sb, bqkv_d[:])

            qk8 = {}
            for s in range(S):
                for w8 in ("q", "k"):
                    for j in range(2):
                        qk8[(s, w8, j)] = resid.tile(
                            [P, 2, NPIX], F8, tag="qk8", bufs=S * 4,
                            name=f"{w8}8_{s}_{j}")
            # vT pair tiles: key (s, tc, jp); [t-part, jj%2, co],
            # pixel m = 512*tc + 4p + (2*jp + jj%2); co = folded proj-out ch
            vT8_sb = {}
            for s in range(S):
                for tcn in range(VTC):
                    for jp in range(2):
                        vT8_sb[(s, tcn, jp)] = resid.tile(
                            [P, 2, C], F8, tag="vt", bufs=S * VTC * 2,
                            name=f"vt8_{s}_{tcn}_{jp}")

            uvpool = tc.alloc_tile_pool(name="uv", bufs=1)
            uv_sb = uvpool.tile([P, NP6 * KY * 2 * 2 * CC * P], F8, tag="uv",
                                bufs=1, name="uv")
            uv_vw = uv_sb.rearrange("p (slot i co) -> p slot i co",
                                    slot=NP6 * KY * 2, i=2, co=C)

            upool = tc.alloc_tile_pool(name="uw", bufs=1)
            u_sb = {}
            u_vw = {}
            for oc in range(QKCH):
                t = upool.tile([P, UCH], F8, tag="u", bufs=4, name=f"u_{oc}")
                u_sb[oc] = t
                u_vw[oc] = t.rearrange("p (slot i co) -> p slot i co",
                                       slot=NP6 * KY * 2, i=2, co=P)

            UVC = KY * 2 * 2 * P   # bytes/partition of one v-position of uw
            UVV = KY * 2 * 2 * C   # bytes/partition of one v-position of uv
            nc.sync.dma_start(v_in[(0, 0)], vw_d[0, 0])
            nc.sync.dma_start(v_in[(0, 1)], vw_d[0, 1])
            nc.gpsimd.dma_start(u_sb[0][:, 0:UVC], uw_d[0][:, 0:UVC])
            nc.gpsimd.dma_start(u_sb[0][:, UVC:2 * UVC], uw_d[0][:, UVC:2 * UVC])
            for v in range(1, NP6):
                for j in range(2):
                    nc.gpsimd.dma_start(v_in[(v, j)], vw_d[v, j])
                if v < NP6 - 1:
                    nc.gpsimd.dma_start(
                        u_sb[0][:, (v + 1) * UVC:(v + 2) * UVC],
                        uw_d[0][:, (v + 1) * UVC:(v + 2) * UVC])
            for v in range(2, 4):
                nc.scalar.dma_start(uv_sb[:, v * UVV:(v + 1) * UVV],
                                    uv_d[:, v * UVV:(v + 1) * UVV])
            for v in range(4, NP6):
                nc.sync.dma_start(uv_sb[:, v * UVV:(v + 1) * UVV],
                                  uv_d[:, v * UVV:(v + 1) * UVV])
            for oc in (1, 2):
                nc.gpsimd.dma_start(u_sb[oc], uw_d[oc])
            for v in range(2):
                nc.gpsimd.dma_start(uv_sb[:, v * UVV:(v + 1) * UVV],
                                    uv_d[:, v * UVV:(v + 1) * UVV])
            for oc in range(3, QKCH):
                nc.gpsimd.dma_start(u_sb[oc], uw_d[oc])

            def drain6(pl, emit):
                # pl[v] = [P, 512] PSUM views of the six A^T input planes;
                # rows {1,1,1,1,1,0; 0,1,-1,2,-2,0; 0,1,1,4,4,0;
                # 0,1,-1,8,-8,1}; ScalarE stages M1, DVE stages M3 (one
                # PSUM operand per DVE op); emit(j, o_bf16[P,512])
                n_ = NTILE
                c1 = stage.tile([P, n_], F32, tag="c1", name=f"c1_{drain6.n}")
                c3 = stage.tile([P, n_], F32, tag="c3", name=f"c3_{drain6.n}")
                sp = stage.tile([P, n_], F32, tag="sp", name=f"sp_{drain6.n}")
                sm = stage.tile([P, n_], F32, tag="sm", name=f"sm_{drain6.n}")
                p34 = stage.tile([P, n_], F32, tag="p34", name=f"p34_{drain6.n}")
                m34 = stage.tile([P, n_], F32, tag="m34", name=f"m34_{drain6.n}")
                t0 = stage.tile([P, n_], F32, tag="t0", name=f"t0_{drain6.n}")
                t3 = stage.tile([P, n_], F32, tag="t3", name=f"t3_{drain6.n}")
                o = [stage.tile([P, n_], BF16, tag=f"o{j}",
                                name=f"o{j}_{drain6.n}") for j in range(4)]
                drain6.n += 1
                nc.scalar.copy(out=c1, in_=pl[1])
                nc.vector.tensor_copy(out=c3, in_=pl[3])
                nc.vector.tensor_add(p34, c3, pl[4])
                nc.vector.tensor_add(sp, c1, pl[2])
                nc.vector.tensor_add(t0, pl[0], sp)
                nc.vector.tensor_add(o[0], t0, p34)
                emit(0, o[0])
                nc.vector.tensor_sub(sm, c1, pl[2])
                nc.vector.tensor_sub(m34, c3, pl[4])
                nc.vector.scalar_tensor_tensor(o[1], m34, 2.0, sm,
                                               op0=MULT, op1=ADD)
                emit(1, o[1])
                nc.vector.scalar_tensor_tensor(o[2], p34, 4.0, sp,
                                               op0=MULT, op1=ADD)
                emit(2, o[2])
                nc.vector.scalar_tensor_tensor(t3, m34, 8.0, pl[5],
                                               op0=MULT, op1=ADD)
                nc.vector.tensor_add(o[3], t3, sm)
                emit(3, o[3])
            drain6.n = 0

            # ONE PSUM pool for the whole kernel: 8 rotating 1-bank tiles.
            mpool = tc.alloc_tile_pool(name="mconv", bufs=1, space="PSUM")

            def bank(name):
                return mpool.tile([P, 2, 256], F32, tag="m", bufs=8, name=name)

            # HAM pre-warm: the PE would otherwise idle ~7us waiting for the
            # first weight DMA and start the conv at K=4/8 (half clock).
            # A burst of dummy matmuls on the memset constants keeps the
            # activity monitor busy so qk0 runs at 2.4 GHz from its first MM.
            warm_ps = bank("warm_ps")
            wrow = warm_ps.rearrange("p a n -> p (a n)")[0:1, :]
            for w in range(24):
                nc.tensor.matmul(wrow, lhsT=ones_nt[:, 0:1], rhs=ones_nt,
                                 start=True, stop=True)

            # q/k conv: plane v in one bank [P, s(2), 256], both samples per
            # matmul (the ky window is contiguous in the flat (y,t) dim)
            def qk_group(oc):
                pls = [bank(f"m_{oc}_{v}") for v in range(NP6)]
                plv = [t.rearrange("p a n -> p (a n)") for t in pls]
                if with_bias:
                    nc.tensor.matmul(plv[1], lhsT=bias_sb[0:1, oc],
                                     rhs=ones_nt[:, 0:NTILE],
                                     start=True, stop=False)
                for v in range(NP6):
                    for ky in range(KY):
                        for j in range(2):
                            nc.tensor.matmul(
                                pls[v],
                                lhsT=u_vw[oc][:, (v * KY + ky) * 2 + j],
                                rhs=v_qk[(v, j)][:, :, :,
                                                 ky * TX:ky * TX + H * TX],
                                start=((v != 1 or not with_bias)
                                       and ky == 0 and j == 0),
                                stop=(ky == KY - 1 and j == 1),
                                perf_mode=DR)
                if oc < CC:
                    w8, ii = "q", oc
                else:
                    w8, ii = "k", oc - CC

                def emit_qk(j, o, w8=w8, ii=ii):
                    ov = o.rearrange("p (s y tx) -> p s y tx", s=S, tx=TX)
                    for s in range(S):
                        dvv = qk8[(s, w8, ii // 2)].rearrange(
                            "p a (y tx four) -> p a y tx four", four=4, tx=TX)
                        nc.scalar.copy(dvv[:, ii % 2, :, :, j], ov[:, s])
                drain6(plv, emit_qk)

            # v conv: operands swapped -> M = [t(128), co]; both co halves
            # share one matmul (rhs [P, i, co512]); weights carry the folded
            # 1x1 proj. Generator yields after each (v, ky) unit so scores
            # chunks can be woven between units.
            def v_units():
                for s, tcn in [(s, t) for s in range(S) for t in range(VTC)]:
                    pls = [bank(f"mv_{s}_{tcn}_{v}") for v in range(NP6)]
                    plv = [t.rearrange("p a n -> p (a n)") for t in pls]
                    for v in range(NP6):
                        for ky in range(KY):
                            off = s * RFL + (16 * tcn + ky) * TX
                            for j in range(2):
                                nc.tensor.matmul(
                                    plv[v],
                                    lhsT=v_fl[(v, j)][:, :, off:off + P],
                                    rhs=uv_vw[:, (v * KY + ky) * 2 + j],
                                    start=(ky == 0 and j == 0),
                                    stop=(ky == KY - 1 and j == 1),
                                    perf_mode=DR)
                            yield v

                    def emit_v(j, o, s=s, tcn=tcn):
                        nc.scalar.mul(vT8_sb[(s, tcn, j // 2)][:, j % 2, :],
                                      o, VEMIT)
                    drain6(plv, emit_v)

            for oc in range(QKCH):
                qk_group(oc)

            upool.release()

            # ---- attention (reuses the released u-weight SBUF zone; the
            # zone dep is on the last qk LDWEIGHTS, already behind us) ----
            attn = tc.alloc_tile_pool(name="attn", bufs=1)
            exps8 = {}
            for s in range(S):
                for tcn in range(VTC):
                    for jp in range(2):
                        exps8[(s, tcn, jp)] = attn.tile(
                            [P, 2, NPIX], F8, tag="exps",
                            bufs=S * VTC * 2, name=f"exps_{s}_{tcn}_{jp}")
            r_sb = {}
            rb_bc = {}
            for s in range(S):
                r_sb[s] = attn.tile([1, NPIX], F32, tag="r", bufs=S,
                                    name=f"r_{s}")
                rb_bc[s] = attn.tile([P, NPIX], F32, tag="rbcb", bufs=S,
                                     name=f"rbcb_{s}")

            def scores_chunk(s, tcn, jp, j2):
                jj = jp * 2 + j2
                ps_nt = [bank(f"ps_sc_{s}_{tcn}_{jj}_{nt}") for nt in range(NT)]
                for j in range(2):
                    kv = qk8[(s, "k", j)].rearrange(
                        "p a (m four) -> p a m four", four=4)
                    lhsT = kv[:, :, P * tcn:P * (tcn + 1), jj]
                    for nt in range(NT):
                        nc.tensor.matmul(
                            ps_nt[nt].rearrange("p a n -> p (a n)"),
                            lhsT=lhsT,
                            rhs=qk8[(s, "q", j)][
                                :, :, nt * NTILE:(nt + 1) * NTILE],
                            start=(j == 0), stop=(j == 1),
                            perf_mode=DR)
                for nt in range(NT):
                    nc.scalar.activation(
                        exps8[(s, tcn, jp)][:, j2, nt * NTILE:(nt + 1) * NTILE],
                        ps_nt[nt].rearrange("p a n -> p (a n)"),
                        EXP, scale=float(C) ** -0.5 / (SU * SU))

            def sums_block(s):
                keys = [(tcn, jp) for tcn in range(VTC) for jp in range(2)]
                for nt in range(NT):
                    ps_sum = bank(f"ps_sum_{s}_{nt}")
                    row = ps_sum.rearrange("p a n -> p (a n)")[0:1, :]
                    for ki, (tcn, jp) in enumerate(keys):
                        nc.tensor.matmul(
                            row, lhsT=ones8[:, :, 0:1],
                            rhs=exps8[(s, tcn, jp)][:, :,
                                                    nt * NTILE:(nt + 1) * NTILE],
                            start=(ki == 0), stop=(ki == len(keys) - 1),
                            perf_mode=DR)
                    nc.vector.reciprocal_approx_fast(
                        out=r_sb[s][:, nt * NTILE:(nt + 1) * NTILE], in_=row)
                nc.gpsimd.partition_broadcast(rb_bc[s], r_sb[s])

            def out_block(s, och):
                # attn@v with proj-folded v: emits final output channels
                keys = [(tcn, jp) for tcn in range(VTC) for jp in range(2)]
                ps_h = [bank(f"ps_h_{s}_{och}_{nt}") for nt in range(NT)]
                for ki, (tcn, jp) in enumerate(keys):
                    lhsT = vT8_sb[(s, tcn, jp)][:, :, och * P:(och + 1) * P]
                    for nt in range(NT):
                        nc.tensor.matmul(
                            ps_h[nt].rearrange("p a n -> p (a n)"), lhsT=lhsT,
                            rhs=exps8[(s, tcn, jp)][
                                :, :, nt * NTILE:(nt + 1) * NTILE],
                            start=(ki == 0), stop=(ki == len(keys) - 1),
                            perf_mode=DR)
                for nt in range(NT):
                    ot = attn.tile([P, NTILE], BF16, tag="ost", bufs=4,
                                   name=f"ot_{s}_{och}_{nt}")
                    nc.vector.scalar_tensor_tensor(
                        ot, ps_h[nt].rearrange("p a n -> p (a n)"), RSC,
                        rb_bc[s][:, nt * NTILE:(nt + 1) * NTILE],
                        op0=MULT, op1=MULT)
                    eng = nc.sync if (och + nt) % 2 == 0 else nc.scalar
                    eng.dma_start(
                        out_d[s, :, och, nt * NTILE:(nt + 1) * NTILE], ot)

            # weave: the 72 v-conv (v,ky) units fill the PE while the
            # ScalarE exp ACTIVATEs pace the 16 scores chunks
            vu = v_units()

            def take(n):
                for _ in range(n):
                    if next(vu, None) is None:
                        break

            sc_keys = [(s, tcn, jp, j2) for s in range(S) for tcn in range(VTC)
                       for jp in range(2) for j2 in range(2)]
            for c, (s, tcn, jp, j2) in enumerate(sc_keys):
                take(5 if c % 2 else 4)
                if c == 15:
                    take(100)  # flush the last v-group drain ahead of sc15
                scores_chunk(s, tcn, jp, j2)
                if c == 8:
                    sums_block(0)
                if 12 <= c < 15:
                    out_block(0, c - 12)
            out_block(0, 3)
            sums_block(1)

            # s=1 outputs: accumulate ki-major across all 8 banks so the
            # first two key rounds (vT from v(1,0)) stream while the last
            # v-group's drain finishes producing vT(1,1); the reciprocal
            # broadcast for s=1 rides between rounds so its serial chain
            # (sum MMs -> recip -> bf16 copy -> matmul) hides under them
            keys = [(tcn, jp) for tcn in range(VTC) for jp in range(2)]
            ps_o1 = {(och, nt): bank(f"ps_o1_{och}_{nt}")
                     for och in range(CC) for nt in range(NT)}
            for ki, (tcn, jp) in enumerate(keys):
                for och in range(CC):
                    lhsT = vT8_sb[(1, tcn, jp)][:, :, och * P:(och + 1) * P]
                    for nt in range(NT):
                        nc.tensor.matmul(
                            ps_o1[(och, nt)].rearrange("p a n -> p (a n)"),
                            lhsT=lhsT,
                            rhs=exps8[(1, tcn, jp)][
                                :, :, nt * NTILE:(nt + 1) * NTILE],
                            start=(ki == 0), stop=(ki == len(keys) - 1),
                            perf_mode=DR)
            for och in range(CC):
                for nt in range(NT):
                    ot = attn.tile([P, NTILE], BF16, tag="ost", bufs=4,
                                   name=f"ot1_{och}_{nt}")
                    nc.vector.scalar_tensor_tensor(
                        ot, ps_o1[(och, nt)].rearrange("p a n -> p (a n)"),
                        RSC, rb_bc[1][:, nt * NTILE:(nt + 1) * NTILE],
                        op0=MULT, op1=MULT)
                    eng = nc.sync if (och + nt) % 2 == 0 else nc.scalar
                    eng.dma_start(
                        out_d[1, :, och, nt * NTILE:(nt + 1) * NTILE], ot)

            mpool.release()
            attn.release()
            uvpool.release()

    nc.finalize()
    return nc


BT43 = np.array([
    [4, 0, -5, 0, 1, 0],
    [0, -4, -4, 1, 1, 0],
    [0, 4, -4, -1, 1, 0],
    [0, -2, -1, 2, 1, 0],
    [0, 2, -1, -2, 1, 0],
    [0, 4, 0, -5, 0, 1]], np.float32)
G43 = np.array([
    [1 / 4, 0, 0],
    [-1 / 6, -1 / 6, -1 / 6],
    [-1 / 6, 1 / 6, -1 / 6],
    [1 / 24, 1 / 12, 1 / 6],
    [1 / 24, -1 / 12, 1 / 6],
    [0, 0, 1]], np.float32)


def prep_inputs(x, w_qkv, b_qkv, w_proj):
    e4 = ml_dtypes.float8_e4m3
    xpad = np.zeros((B, C, HP, WP), np.float32)
    xpad[:, :, 1:H + 1, 1:W + 1] = x

    taps = np.stack([xpad[:, :, :, a:a + 4 * TX:4][:, :, :, :TX]
                     for a in range(6)])          # [6, B, C, HP, TX]
    V = np.tensordot(BT43, taps, axes=([1], [0]))  # [6, B, C, HP, TX]
    vw = np.ascontiguousarray(
        V.reshape(NP6, B, 2, 2, P, HP, TX)
        .transpose(0, 2, 4, 3, 1, 5, 6)).astype(e4)  # [6, 2, P, 2, B, HP, TX]

    u6qk = np.tensordot(w_qkv[:2 * C] * SU, G43, axes=([3], [1]))
    uw = np.ascontiguousarray(
        u6qk.reshape(QKCH, P, 2, 2, P, KY, NP6)
        .transpose(0, 4, 6, 5, 2, 3, 1)
        .reshape(QKCH, P, UCH)).astype(e4)
    # fold the 1x1 proj into the v weights: conv(x, Wp @ wv) == proj(conv(x, wv))
    u6v = np.tensordot(w_qkv[2 * C:], G43, axes=([3], [1]))  # [vc, ci, ky, 6]
    u6vp = np.tensordot(w_proj[:, :, 0, 0], u6v, axes=([1], [0])) * SWV
    uv = np.ascontiguousarray(
        u6vp.reshape(C, 2, 2, P, KY, NP6)
        .transpose(3, 5, 4, 1, 2, 0)
        .reshape(P, NP6 * KY * 2 * 2 * C)).astype(e4)
    bqkv = np.ascontiguousarray((b_qkv[:2 * C] * SU).reshape(QKCH, P)).astype(
        ml_dtypes.bfloat16)
    return vw, uw, uv, bqkv


def kernel(x, w_qkv, b_qkv, w_proj, b_proj, gn_gamma=None, gn_beta=None):
    global LAST_EXEC_NS
    x = np.asarray(x, np.float32)
    w_qkv = np.asarray(w_qkv, np.float32)
    b_qkv = np.asarray(b_qkv, np.float32)
    w_proj = np.asarray(w_proj, np.float32)
    b_proj = np.asarray(b_proj, np.float32)

    with_bias = bool(np.any(b_qkv[:2 * C]))
    if with_bias not in _CACHED:
        _CACHED[with_bias] = build_nc(with_bias=with_bias)
    nc = _CACHED[with_bias]

    vw, uw, uv, bqkv = prep_inputs(x, w_qkv, b_qkv, w_proj)

    in_maps = []
    for core in range(NCORES):
        sl = slice(core * S, (core + 1) * S)
        in_maps.append({
            "vw": np.ascontiguousarray(vw[:, :, :, :, sl]).reshape(NP6, 2, P, -1),
            "uw": uw,
            "uv": uv,
            "bqkv": bqkv,
        })

    res = run_bass_kernel_spmd(nc, in_maps, list(range(NCORES)), trace=TRACE)
    LAST_EXEC_NS = res.exec_time_ns
    h = np.stack([np.asarray(res.results[c]["out"], np.float32)
                  for c in range(NCORES)])
    h = h.reshape(B, P, CC, NPIX).transpose(0, 2, 1, 3).reshape(B, C, H, W)
    out = x + h + b_proj[None, :, None, None]
    # v-bias passes through softmax-weighting as a constant channel offset
    bv = b_qkv[2 * C:]
    if np.any(bv):
        out = out + (w_proj[:, :, 0, 0] @ bv)[None, :, None, None]
    return np.ascontiguousarray(out).astype(np.float32, copy=False)


# revision 17
# speedup vs baseline: 1.0596x; 1.0052x over previous
"""AttnBlock on 8 TRN2 NeuronCores -- F(4,3) x-Winograd, folded-proj variant.

Same conv math as the F(4,3) baseline (host-side input/weight transforms,
fp8 DoubleRow GEMMs with fp32 PSUM, v produced pre-transposed by swapping
conv operands, deferred softmax normalization), restructured for overlap:

  - the 1x1 proj conv is folded into the v-conv weights on the host
    (both are linear maps over channels), so attn@v emits the final
    output channels directly: no hN intermediate, no proj GEMM stage.
    The tiny w_proj magnitudes (~1e-7) are scaled up by 2^26 into fp8
    range; the 2^-20 net factor rides the reciprocal broadcast.
  - merged matmuls: the two per-core samples share one qk matmul
    (rhs [P, i, s, 256], the ky window is contiguous in the flat (y,t)
    dim) and the two v-conv output halves share one matmul (rhs
    [P, i, co512]). Halves MATMUL+LDWEIGHTS instruction count and
    widens the A^T drain from 256 to 512 columns (fewer, cheaper DVE
    ops). PSUM banks hold one winograd plane for both samples/halves.
  - ONE PSUM tile pool (8 one-bank bufs, single tag) spans conv and
    attention: pool release/realloc at the phase boundary is a full
    barrier (released-zone alloc deps) that idled the PE ~8us and
    re-throttled HAM to K=4/8 for the attention phase.
  - the v-conv units are WOVEN between scores chunks: scores are paced
    by the ScalarE exp ACTIVATEs (~1.25us per chunk vs 0.97us of MM),
    so pure scores blocks stall the PE; v matmuls fill those slots.
  - initial DMAs: the gpsimd software queue sustains ~200+ GB/s, the
    sync/scalar hardware queues only ~40 GB/s. The conv-critical
    stream (u0 per-v chunks + v_in tiles in consumption order, then
    u1..u7) rides gpsimd; the slow queues prefetch the v-conv weights
    (not needed until ~70us in) and later carry the output tiles.
  - drain staging copies split ScalarE (c1) / DVE (c3).
"""

import numpy as np
import ml_dtypes

import concourse.bass as bass
import concourse.tile as tile
from concourse import bacc, mybir
from concourse.bass_utils import run_bass_kernel_spmd

P = 128
B, C, H, W = 16, 512, 32, 32
NCORES = 8
S = B // NCORES
HP = WP = H + 2
NPIX = H * W
CC = C // P
QKCH = 2 * CC
NT = 2
NTILE = 512
TX = W // 4          # 8 x-tiles
NP6 = 6              # winograd positions
KY = 3
NQK = H * TX         # 256: per-sample N for q/k GEMMs
VTC = 2              # v-conv t-chunks per sample (256 t / 128)
SU = 32.0            # q/k weight scale into fp8
SWV = float(2 ** 26)  # folded v*proj weight scale into fp8
VEMIT = 1.0 / 64     # v-conv PSUM -> fp8 emit scale
RSC = 2.0 ** -20     # net 1/(SWV*VEMIT) rides the reciprocal broadcast

BF16 = mybir.dt.bfloat16
F32 = mybir.dt.float32
F8 = mybir.dt.float8e4
EXP = mybir.ActivationFunctionType.Exp
DR = mybir.MatmulPerfMode.DoubleRow
MULT = mybir.AluOpType.mult
ADD = mybir.AluOpType.add

TRACE = False
LAST_EXEC_NS = None

_CACHED = {}

UCH = NP6 * KY * 2 * 2 * P  # bytes/partition of one q/k weight tile
RFL = HP * TX               # flat (y,t) extent per (i,s)


def build_nc(with_bias=True):
    nc = bacc.Bacc()
    vw_d = nc.declare_dram_parameter("vw", [NP6, 2, P, 2 * S * HP * TX], F8,
                                     isOutput=False)
    uw_d = nc.declare_dram_parameter("uw", [QKCH, P, UCH], F8, isOutput=False)
    uv_d = nc.declare_dram_parameter("uv", [P, NP6 * KY * 2 * 2 * CC * P], F8,
                                     isOutput=False)
    bqkv_d = nc.declare_dram_parameter("bqkv", [QKCH, P], BF16, isOutput=False)
    out_d = nc.declare_dram_parameter("out", [S, P, CC, NPIX], BF16, isOutput=True)

    with tile.TileContext(nc) as tc:
        with (
            tc.tile_pool(name="const", bufs=1) as constp,
            tc.tile_pool(name="resid", bufs=1) as resid,
            tc.tile_pool(name="stage", bufs=2) as stage,
        ):
            ones8 = constp.tile([P, 2, 16], F8, name="ones8")
            nc.vector.memset(ones8, 1.0)
            ones_nt = constp.tile([1, NTILE], BF16, name="ones_nt")
            nc.vector.memset(ones_nt, 1.0)

            v_in = {}
            v_qk = {}   # [p, i, s, flat(y,t)] for merged-sample qk matmuls
            v_fl = {}
            for v in range(NP6):
                for j in range(2):
                    t = resid.tile([P, 2 * S * RFL], F8, tag="vin",
                                   bufs=NP6 * 2, name=f"vin_{v}_{j}")
                    v_in[(v, j)] = t
                    v_qk[(v, j)] = t.rearrange("p (i s r) -> p i s r",
                                               i=2, s=S)
                    v_fl[(v, j)] = t.rearrange("p (i r) -> p i r", i=2)

            bias_sb = constp.tile([1, QKCH, P], BF16, name="bias_sb")
            if with_bias:
                nc.sync.dma_start(bias_sb, bqkv_d[:])

            qk8 = {}
            for s in range(S):
                for w8 in ("q", "k"):
                    for j in range(2):
                        qk8[(s, w8, j)] = resid.tile(
                            [P, 2, NPIX], F8, tag="qk8", bufs=S * 4,
                            name=f"{w8}8_{s}_{j}")
            # vT pair tiles: key (s, tc, jp); [t-part, jj%2, co],
            # pixel m = 512*tc + 4p + (2*jp + jj%2); co = folded proj-out ch
            vT8_sb = {}
            for s in range(S):
                for tcn in range(VTC):
                    for jp in range(2):
                        vT8_sb[(s, tcn, jp)] = resid.tile(
                            [P, 2, C], F8, tag="vt", bufs=S * VTC * 2,
                            name=f"vt8_{s}_{tcn}_{jp}")

            uvpool = tc.alloc_tile_pool(name="uv", bufs=1)
            uv_sb = uvpool.tile([P, NP6 * KY * 2 * 2 * CC * P], F8, tag="uv",
                                bufs=1, name="uv")
            uv_vw = uv_sb.rearrange("p (slot i co) -> p slot i co",
                                    slot=NP6 * KY * 2, i=2, co=C)

            upool = tc.alloc_tile_pool(name="uw", bufs=1)
            u_sb = {}
            u_vw = {}
            for oc in range(QKCH):
                t = upool.tile([P, UCH], F8, tag="u", bufs=4, name=f"u_{oc}")
                u_sb[oc] = t
                u_vw[oc] = t.rearrange("p (slot i co) -> p slot i co",
                                       slot=NP6 * KY * 2, i=2, co=P)

            UVC = KY * 2 * 2 * P   # bytes/partition of one v-position of uw
            UVV = KY * 2 * 2 * C   # bytes/partition of one v-position of uv
            nc.sync.dma_start(v_in[(0, 0)], vw_d[0, 0])
            nc.sync.dma_start(v_in[(0, 1)], vw_d[0, 1])
            nc.gpsimd.dma_start(u_sb[0][:, 0:UVC], uw_d[0][:, 0:UVC])
            nc.gpsimd.dma_start(u_sb[0][:, UVC:2 * UVC], uw_d[0][:, UVC:2 * UVC])
            for v in range(1, NP6):
                for j in range(2):
                    nc.gpsimd.dma_start(v_in[(v, j)], vw_d[v, j])
                if v < NP6 - 1:
                    nc.gpsimd.dma_start(
                        u_sb[0][:, (v + 1) * UVC:(v + 2) * UVC],
                        uw_d[0][:, (v + 1) * UVC:(v + 2) * UVC])
            for v in range(2, 4):
                nc.scalar.dma_start(uv_sb[:, v * UVV:(v + 1) * UVV],
                                    uv_d[:, v * UVV:(v + 1) * UVV])
            for v in range(4, NP6):
                nc.sync.dma_start(uv_sb[:, v * UVV:(v + 1) * UVV],
                                  uv_d[:, v * UVV:(v + 1) * UVV])
            for oc in (1, 2):
                nc.gpsimd.dma_start(u_sb[oc], uw_d[oc])
            for v in range(2):
                nc.gpsimd.dma_start(uv_sb[:, v * UVV:(v + 1) * UVV],
                                    uv_d[:, v * UVV:(v + 1) * UVV])
            for oc in range(3, QKCH):
                nc.gpsimd.dma_start(u_sb[oc], uw_d[oc])

            def drain6(pl, emit):
                # pl[v] = [P, 512] PSUM views of the six A^T input planes;
                # rows {1,1,1,1,1,0; 0,1,-1,2,-2,0; 0,1,1,4,4,0;
                # 0,1,-1,8,-8,1}; ScalarE stages M1, DVE stages M3 (one
                # PSUM operand per DVE op); emit(j, o_bf16[P,512])
                n_ = NTILE
                c1 = stage.tile([P, n_], F32, tag="c1", name=f"c1_{drain6.n}")
                c3 = stage.tile([P, n_], F32, tag="c3", name=f"c3_{drain6.n}")
                sp = stage.tile([P, n_], F32, tag="sp", name=f"sp_{drain6.n}")
                sm = stage.tile([P, n_], F32, tag="sm", name=f"sm_{drain6.n}")
                p34 = stage.tile([P, n_], F32, tag="p34", name=f"p34_{drain6.n}")
                m34 = stage.tile([P, n_], F32, tag="m34", name=f"m34_{drain6.n}")
                t0 = stage.tile([P, n_], F32, tag="t0", name=f"t0_{drain6.n}")
                t3 = stage.tile([P, n_], F32, tag="t3", name=f"t3_{drain6.n}")
                o = [stage.tile([P, n_], BF16, tag=f"o{j}",
                                name=f"o{j}_{drain6.n}") for j in range(4)]
                drain6.n += 1
                nc.scalar.copy(out=c1, in_=pl[1])
                nc.vector.tensor_copy(out=c3, in_=pl[3])
                nc.vector.tensor_add(p34, c3, pl[4])
                nc.vector.tensor_add(sp, c1, pl[2])
                nc.vector.tensor_add(t0, pl[0], sp)
                nc.vector.tensor_add(o[0], t0, p34)
                emit(0, o[0])
                nc.vector.tensor_sub(sm, c1, pl[2])
                nc.vector.tensor_sub(m34, c3, pl[4])
                nc.vector.scalar_tensor_tensor(o[1], m34, 2.0, sm,
                                               op0=MULT, op1=ADD)
                emit(1, o[1])
                nc.vector.scalar_tensor_tensor(o[2], p34, 4.0, sp,
                                               op0=MULT, op1=ADD)
                emit(2, o[2])
                nc.vector.scalar_tensor_tensor(t3, m34, 8.0, pl[5],
                                               op0=MULT, op1=ADD)
                nc.vector.tensor_add(o[3], t3, sm)
                emit(3, o[3])
            drain6.n = 0

            # ONE PSUM pool for the whole kernel: 8 rotating 1-bank tiles.
            mpool = tc.alloc_tile_pool(name="mconv", bufs=1, space="PSUM")

            def bank(name):
                return mpool.tile([P, 2, 256], F32, tag="m", bufs=8, name=name)

            # HAM pre-warm: the PE would otherwise idle ~7us waiting for the
            # first weight DMA and start the conv at K=4/8 (half clock).
            # A burst of dummy matmuls on the memset constants keeps the
            # activity monitor busy so qk0 runs at 2.4 GHz from its first MM.
            warm_ps = [bank("warm_ps0"), bank("warm_ps1")]
            wrow = [t.rearrange("p a n -> p (a n)")[0:1, :] for t in warm_ps]
            for w in range(14):
                nc.tensor.matmul(wrow[w % 2], lhsT=ones_nt[:, 0:1],
                                 rhs=ones_nt, start=True, stop=True)

            # q/k conv: plane v in one bank [P, s(2), 256], both samples per
            # matmul (the ky window is contiguous in the flat (y,t) dim)
            def qk_group(oc):
                pls = [bank(f"m_{oc}_{v}") for v in range(NP6)]
                plv = [t.rearrange("p a n -> p (a n)") for t in pls]
                if with_bias:
                    nc.tensor.matmul(plv[1], lhsT=bias_sb[0:1, oc],
                                     rhs=ones_nt[:, 0:NTILE],
                                     start=True, stop=False)
                for v in range(NP6):
                    for ky in range(KY):
                        for j in range(2):
                            nc.tensor.matmul(
                                pls[v],
                                lhsT=u_vw[oc][:, (v * KY + ky) * 2 + j],
                                rhs=v_qk[(v, j)][:, :, :,
                                                 ky * TX:ky * TX + H * TX],
                                start=((v != 1 or not with_bias)
                                       and ky == 0 and j == 0),
                                stop=(ky == KY - 1 and j == 1),
                                perf_mode=DR)
                if oc < CC:
                    w8, ii = "q", oc
                else:
                    w8, ii = "k", oc - CC

                def emit_qk(j, o, w8=w8, ii=ii):
                    ov = o.rearrange("p (s y tx) -> p s y tx", s=S, tx=TX)
                    for s in range(S):
                        dvv = qk8[(s, w8, ii // 2)].rearrange(
                            "p a (y tx four) -> p a y tx four", four=4, tx=TX)
                        nc.scalar.copy(dvv[:, ii % 2, :, :, j], ov[:, s])
                drain6(plv, emit_qk)

            # v conv: operands swapped -> M = [t(128), co]; both co halves
            # share one matmul (rhs [P, i, co512]); weights carry the folded
            # 1x1 proj. Generator yields after each (v, ky) unit so scores
            # chunks can be woven between units.
            def v_units():
                for s, tcn in [(s, t) for s in range(S) for t in range(VTC)]:
                    pls = [bank(f"mv_{s}_{tcn}_{v}") for v in range(NP6)]
                    plv = [t.rearrange("p a n -> p (a n)") for t in pls]
                    for v in range(NP6):
                        for ky in range(KY):
                            off = s * RFL + (16 * tcn + ky) * TX
                            for j in range(2):
                                nc.tensor.matmul(
                                    plv[v],
                                    lhsT=v_fl[(v, j)][:, :, off:off + P],
                                    rhs=uv_vw[:, (v * KY + ky) * 2 + j],
                                    start=(ky == 0 and j == 0),
                                    stop=(ky == KY - 1 and j == 1),
                                    perf_mode=DR)
                            yield v

                    def emit_v(j, o, s=s, tcn=tcn):
                        nc.scalar.mul(vT8_sb[(s, tcn, j // 2)][:, j % 2, :],
                                      o, VEMIT)
                    drain6(plv, emit_v)

            for oc in range(QKCH):
                qk_group(oc)

            upool.release()

            # ---- attention (reuses the released u-weight SBUF zone; the
            # zone dep is on the last qk LDWEIGHTS, already behind us) ----
            attn = tc.alloc_tile_pool(name="attn", bufs=1)
            exps8 = {}
            for s in range(S):
                for tcn in range(VTC):
                    for jp in range(2):
                        exps8[(s, tcn, jp)] = attn.tile(
                            [P, 2, NPIX], F8, tag="exps",
                            bufs=S * VTC * 2, name=f"exps_{s}_{tcn}_{jp}")
            r_sb = {}
            rb_bc = {}
            for s in range(S):
                r_sb[s] = attn.tile([1, NPIX], F32, tag="r", bufs=S,
                                    name=f"r_{s}")
                rb_bc[s] = attn.tile([P, NPIX], F32, tag="rbcb", bufs=S,
                                     name=f"rbcb_{s}")

            def scores_chunk(s, tcn, jp, j2):
                jj = jp * 2 + j2
                ps_nt = [bank(f"ps_sc_{s}_{tcn}_{jj}_{nt}") for nt in range(NT)]
                for j in range(2):
                    kv = qk8[(s, "k", j)].rearrange(
                        "p a (m four) -> p a m four", four=4)
                    lhsT = kv[:, :, P * tcn:P * (tcn + 1), jj]
                    for nt in range(NT):
                        nc.tensor.matmul(
                            ps_nt[nt].rearrange("p a n -> p (a n)"),
                            lhsT=lhsT,
                            rhs=qk8[(s, "q", j)][
                                :, :, nt * NTILE:(nt + 1) * NTILE],
                            start=(j == 0), stop=(j == 1),
                            perf_mode=DR)
                for nt in range(NT):
                    nc.scalar.activation(
                        exps8[(s, tcn, jp)][:, j2, nt * NTILE:(nt + 1) * NTILE],
                        ps_nt[nt].rearrange("p a n -> p (a n)"),
                        EXP, scale=float(C) ** -0.5 / (SU * SU))

            def sums_block(s):
                keys = [(tcn, jp) for tcn in range(VTC) for jp in range(2)]
                for nt in range(NT):
                    ps_sum = bank(f"ps_sum_{s}_{nt}")
                    row = ps_sum.rearrange("p a n -> p (a n)")[0:1, :]
                    for ki, (tcn, jp) in enumerate(keys):
                        nc.tensor.matmul(
                            row, lhsT=ones8[:, :, 0:1],
                            rhs=exps8[(s, tcn, jp)][:, :,
                                                    nt * NTILE:(nt + 1) * NTILE],
                            start=(ki == 0), stop=(ki == len(keys) - 1),
                            perf_mode=DR)
                    nc.vector.reciprocal_approx_fast(
                        out=r_sb[s][:, nt * NTILE:(nt + 1) * NTILE], in_=row)
                nc.gpsimd.partition_broadcast(rb_bc[s], r_sb[s])

            def out_block(s, och):
                # attn@v with proj-folded v: emits final output channels
                keys = [(tcn, jp) for tcn in range(VTC) for jp in range(2)]
                ps_h = [bank(f"ps_h_{s}_{och}_{nt}") for nt in range(NT)]
                for ki, (tcn, jp) in enumerate(keys):
                    lhsT = vT8_sb[(s, tcn, jp)][:, :, och * P:(och + 1) * P]
                    for nt in range(NT):
                        nc.tensor.matmul(
                            ps_h[nt].rearrange("p a n -> p (a n)"), lhsT=lhsT,
                            rhs=exps8[(s, tcn, jp)][
                                :, :, nt * NTILE:(nt + 1) * NTILE],
                            start=(ki == 0), stop=(ki == len(keys) - 1),
                            perf_mode=DR)
                for nt in range(NT):
                    ot = attn.tile([P, NTILE], BF16, tag="ost", bufs=4,
                                   name=f"ot_{s}_{och}_{nt}")
                    nc.vector.scalar_tensor_tensor(
                        ot, ps_h[nt].rearrange("p a n -> p (a n)"), RSC,
                        rb_bc[s][:, nt * NTILE:(nt + 1) * NTILE],
                        op0=MULT, op1=MULT)
                    eng = nc.sync if (och + nt) % 2 == 0 else nc.scalar
                    eng.dma_start(
                        out_d[s, :, och, nt * NTILE:(nt + 1) * NTILE], ot)

            # weave: the 72 v-conv (v,ky) units fill the PE while the
            # ScalarE exp ACTIVATEs pace the 16 scores chunks
            vu = v_units()

            def take(n):
                for _ in range(n):
                    if next(vu, None) is None:
                        break

            sc_keys = [(s, tcn, jp, j2) for s in range(S) for tcn in range(VTC)
                       for jp in range(2) for j2 in range(2)]
            for c, (s, tcn, jp, j2) in enumerate(sc_keys):
                take(5 if c % 2 else 4)
                if c == 15:
                    take(100)  # flush the last v-group drain ahead of sc15
                scores_chunk(s, tcn, jp, j2)
                if c == 9:
                    sums_block(0)
                if 12 <= c < 15:
                    out_block(0, c - 12)
            out_block(0, 3)
            sums_block(1)

            # s=1 outputs: accumulate ki-major across all 8 banks so the
            # first two key rounds (vT from v(1,0)) stream while the last
            # v-group's drain finishes producing vT(1,1); the reciprocal
            # broadcast for s=1 rides between rounds so its serial chain
            # (sum MMs -> recip -> bf16 copy -> matmul) hides under them
            keys = [(tcn, jp) for tcn in range(VTC) for jp in range(2)]
            ps_o1 = {(och, nt): bank(f"ps_o1_{och}_{nt}")
                     for och in range(CC) for nt in range(NT)}
            for ki, (tcn, jp) in enumerate(keys):
                for och in range(CC):
                    lhsT = vT8_sb[(1, tcn, jp)][:, :, och * P:(och + 1) * P]
                    for nt in range(NT):
                        nc.tensor.matmul(
                            ps_o1[(och, nt)].rearrange("p a n -> p (a n)"),
                            lhsT=lhsT,
                            rhs=exps8[(1, tcn, jp)][
                                :, :, nt * NTILE:(nt + 1) * NTILE],
                            start=(ki == 0), stop=(ki == len(keys) - 1),
                            perf_mode=DR)
            for och in range(CC):
                for nt in range(NT):
                    ot = attn.tile([P, NTILE], BF16, tag="ost", bufs=4,
                                   name=f"ot1_{och}_{nt}")
                    nc.vector.scalar_tensor_tensor(
                        ot, ps_o1[(och, nt)].rearrange("p a n -> p (a n)"),
                        RSC, rb_bc[1][:, nt * NTILE:(nt + 1) * NTILE],
                        op0=MULT, op1=MULT)
                    eng = nc.sync if (och + nt) % 2 == 0 else nc.scalar
                    eng.dma_start(
                        out_d[1, :, och, nt * NTILE:(nt + 1) * NTILE], ot)

            mpool.release()
            attn.release()
            uvpool.release()

    nc.finalize()
    return nc


BT43 = np.array([
    [4, 0, -5, 0, 1, 0],
    [0, -4, -4, 1, 1, 0],
    [0, 4, -4, -1, 1, 0],
    [0, -2, -1, 2, 1, 0],
    [0, 2, -1, -2, 1, 0],
    [0, 4, 0, -5, 0, 1]], np.float32)
G43 = np.array([
    [1 / 4, 0, 0],
    [-1 / 6, -1 / 6, -1 / 6],
    [-1 / 6, 1 / 6, -1 / 6],
    [1 / 24, 1 / 12, 1 / 6],
    [1 / 24, -1 / 12, 1 / 6],
    [0, 0, 1]], np.float32)


def prep_inputs(x, w_qkv, b_qkv, w_proj):
    e4 = ml_dtypes.float8_e4m3
    xpad = np.zeros((B, C, HP, WP), np.float32)
    xpad[:, :, 1:H + 1, 1:W + 1] = x

    taps = np.stack([xpad[:, :, :, a:a + 4 * TX:4][:, :, :, :TX]
                     for a in range(6)])          # [6, B, C, HP, TX]
    V = np.tensordot(BT43, taps, axes=([1], [0]))  # [6, B, C, HP, TX]
    vw = np.ascontiguousarray(
        V.reshape(NP6, B, 2, 2, P, HP, TX)
        .transpose(0, 2, 4, 3, 1, 5, 6)).astype(e4)  # [6, 2, P, 2, B, HP, TX]

    u6qk = np.tensordot(w_qkv[:2 * C] * SU, G43, axes=([3], [1]))
    uw = np.ascontiguousarray(
        u6qk.reshape(QKCH, P, 2, 2, P, KY, NP6)
        .transpose(0, 4, 6, 5, 2, 3, 1)
        .reshape(QKCH, P, UCH)).astype(e4)
    # fold the 1x1 proj into the v weights: conv(x, Wp @ wv) == proj(conv(x, wv))
    u6v = np.tensordot(w_qkv[2 * C:], G43, axes=([3], [1]))  # [vc, ci, ky, 6]
    u6vp = np.tensordot(w_proj[:, :, 0, 0], u6v, axes=([1], [0])) * SWV
    uv = np.ascontiguousarray(
        u6vp.reshape(C, 2, 2, P, KY, NP6)
        .transpose(3, 5, 4, 1, 2, 0)
        .reshape(P, NP6 * KY * 2 * 2 * C)).astype(e4)
    bqkv = np.ascontiguousarray((b_qkv[:2 * C] * SU).reshape(QKCH, P)).astype(
        ml_dtypes.bfloat16)
    return vw, uw, uv, bqkv


def kernel(x, w_qkv, b_qkv, w_proj, b_proj, gn_gamma=None, gn_beta=None):
    global LAST_EXEC_NS
    x = np.asarray(x, np.float32)
    w_qkv = np.asarray(w_qkv, np.float32)
    b_qkv = np.asarray(b_qkv, np.float32)
    w_proj = np.asarray(w_proj, np.float32)
    b_proj = np.asarray(b_proj, np.float32)

    with_bias = bool(np.any(b_qkv[:2 * C]))
    if with_bias not in _CACHED:
        _CACHED[with_bias] = build_nc(with_bias=with_bias)
    nc = _CACHED[with_bias]

    vw, uw, uv, bqkv = prep_inputs(x, w_qkv, b_qkv, w_proj)

    in_maps = []
    for core in range(NCORES):
        sl = slice(core * S, (core + 1) * S)
        in_maps.append({
            "vw": np.ascontiguousarray(vw[:, :, :, :, sl]).reshape(NP6, 2, P, -1),
            "uw": uw,
            "uv": uv,
            "bqkv": bqkv,
        })

    res = run_bass_kernel_spmd(nc, in_maps, list(range(NCORES)), trace=TRACE)
    LAST_EXEC_NS = res.exec_time_ns
    h = np.stack([np.asarray(res.results[c]["out"], np.float32)
                  for c in range(NCORES)])
    h = h.reshape(B, P, CC, NPIX).transpose(0, 2, 1, 3).reshape(B, C, H, W)
    out = x + h + b_proj[None, :, None, None]
    # v-bias passes through softmax-weighting as a constant channel offset
    bv = b_qkv[2 * C:]
    if np.any(bv):
        out = out + (w_proj[:, :, 0, 0] @ bv)[None, :, None, None]
    return np.ascontiguousarray(out).astype(np.float32, copy=False)
